# revision 1
# baseline (speedup 1.0000x reference)
"""Bass/Trainium2 kernel for nn_Encoder (embedding -> BiLSTM -> cross attention -> enhancement).

Sharding: data-parallel over batch, 16 items per core on 8 NeuronCores
(per the data-parallel hint; no collectives needed). Per core the A and B
sequences are stacked into 32 rows and the fwd/bwd LSTM directions run as two
interleaved dependency chains sharing the engines.

Phases per core: (1) input projections x@Wih^T+bias for both directions as
dense f32r matmuls staged to DRAM, (2) the 128-step recurrent scan — per step
and direction 16 h^T@Whh^T f32r matmuls into PSUM, per-bank DVE adds of the
staged xw, per-gate in-place activations in [g,i,f,o] order (so tanh(g)
starts after the first PSUM bank), cell/hidden elementwise, and a PE
transpose producing the next step's stationary h^T, (3) cross-attention:
PE transposes to feature-major, E/E^T f32 matmuls, row softmaxes via
Exp-with-accum, soft alignments as f32r matmuls, and the 4-way enhancement
concat streamed straight to the outputs.

float32r (full-rate fp32 PE mode, ~1e-4 matmul rel err) is used for all
large-N matmuls; elementwise math and the small-N attention logit matmuls
stay fp32.
"""

import numpy as np

V, E, H = 32000, 300, 512
BSZ, T = 128, 128
NCORES = 8
PB = BSZ // NCORES          # 16 batch items per core
RW = 2 * PB                 # 32 stacked rows (A items then B items)
RT = 2 * RW                 # 64 rows in fused fwd+bwd elementwise space
G4 = 4 * H                  # 2048 gate width
H2 = 2 * H                  # 1024 bilstm output width
KCH = [(0, 128), (128, 128), (256, 44)]   # chunks of E=300

_CACHE = {}


def _build(phases=3, scan_T=T, xwp_bufs=2, gp_bufs=2, a3_bufs=2, eps_bufs=2):
    import concourse.mybir as mybir
    import concourse.tile as tile
    from concourse import bacc
    from concourse.masks import make_identity

    F32 = mybir.dt.float32
    F32R = mybir.dt.float32r
    F16 = mybir.dt.float16
    AF = mybir.ActivationFunctionType
    ALU = mybir.AluOpType
    AX = mybir.AxisListType

    nc = bacc.Bacc("TRN2", target_bir_lowering=False, debug=False,
                   num_devices=NCORES)

    xT_d = nc.dram_tensor("xT", [E, RW * T], F32R, kind="ExternalInput")
    wih_d = {d: nc.dram_tensor(f"wihT_{d}", [E, G4], F32R, kind="ExternalInput")
             for d in "fb"}
    whh_d = {d: nc.dram_tensor(f"whhT_{d}", [H, G4], F32R, kind="ExternalInput")
             for d in "fb"}
    bias_d = {d: nc.dram_tensor(f"bias_{d}", [128, G4], F32, kind="ExternalInput")
              for d in "fb"}
    outA_d = nc.dram_tensor("outA", [PB, T, 4 * H2], F32, kind="ExternalOutput")
    outB_d = nc.dram_tensor("outB", [PB, T, 4 * H2], F32, kind="ExternalOutput")

    with tile.TileContext(nc) as tc:
        with tc.tile_pool(name="dram", bufs=1, space="DRAM") as dpool, \
             tc.tile_pool(name="const", bufs=1) as const:
            xw = {d: dpool.tile([RW, T, G4], F16, name=f"xw_{d}") for d in "fb"}
            tm = dpool.tile([RW, T, H2], F32R)
            ident = const.tile([128, 128], F32)
            make_identity(nc, ident[:])
            identr = const.tile([128, 128], F32R)
            nc.vector.tensor_copy(identr[:], ident[:])
            ident16 = const.tile([128, 128], F16)
            nc.vector.tensor_copy(ident16[:], ident[:])

            # ---------------- Phase 1: input projections ----------------
            with tc.tile_pool(name="p1w", bufs=1) as p1w, \
                 tc.tile_pool(name="p1ps", bufs=2, space="PSUM") as p1ps, \
                 tc.tile_pool(name="p1e", bufs=3) as p1e:
                xT_sb = []
                for ki, (ko, ks) in enumerate(KCH):
                    t_ = p1w.tile([ks, RW * T], F32R, tag=f"xT{ki}")
                    nc.sync.dma_start(t_[:], xT_d.ap()[ko:ko + ks, :])
                    xT_sb.append(t_)
                for d in "fb":
                    wih_sb = []
                    for ki, (ko, ks) in enumerate(KCH):
                        t_ = p1w.tile([ks, G4], F32R, tag=f"wih{d}{ki}")
                        nc.sync.dma_start(t_[:], wih_d[d].ap()[ko:ko + ks, :])
                        wih_sb.append(t_)
                    bias_sb = p1w.tile([128, G4], F32, tag=f"bias{d}")
                    nc.sync.dma_start(bias_sb[:], bias_d[d].ap())
                    for rc in range(RW):
                        ps = p1ps.tile([128, G4], F32, tag="pj")
                        for nj in range(4):
                            for ki in range(3):
                                nc.tensor.matmul(
                                    ps[:, nj * 512:(nj + 1) * 512],
                                    xT_sb[ki][:, rc * T:(rc + 1) * T],
                                    wih_sb[ki][:, nj * 512:(nj + 1) * 512],
                                    start=(ki == 0), stop=(ki == 2))
                        ev = p1e.tile([128, G4], F16, tag="ev")
                        nc.vector.tensor_add(ev[:], ps[:], bias_sb[:])
                        nc.sync.dma_start(xw[d][rc, :, :], ev[:])

            # ---------------- Phase 2: bidirectional LSTM scan ----------------
            if phases < 2:
                nc.compile()
                return nc
            with tc.tile_pool(name="wst", bufs=1) as wst, \
                 tc.tile_pool(name="sst", bufs=1) as sst, \
                 tc.tile_pool(name="xwp", bufs=xwp_bufs) as xwp, \
                 tc.tile_pool(name="gp", bufs=gp_bufs) as gp, \
                 tc.tile_pool(name="gps", bufs=1, space="PSUM") as gps_pool, \
                 tc.tile_pool(name="tps", bufs=2, space="PSUM") as tps_pool:
                whh_sb = {}
                for d in "fb":
                    whh_sb[d] = []
                    for kc in range(4):
                        w = wst.tile([128, G4], F32R, tag=f"whh{d}{kc}")
                        nc.sync.dma_start(w[:], whh_d[d].ap()[kc * 128:(kc + 1) * 128, :])
                        whh_sb[d].append(w)
                # hT_d: transposed h state per direction; chunk c in cols [32c:32c+32]
                hT = {d: sst.tile([128, 4 * RW], F32R, name=f"hT_{d}") for d in "fb"}
                c_st = {d: sst.tile([RW, H], F32, name=f"c_st_{d}") for d in "fb"}

                # gates layout (host permuted): [g | i | f | o]
                GG, GI, GF, GO = 0, 1, 2, 3
                for t in range(scan_T):
                    for di, d in enumerate("fb"):
                        tx = t if d == "f" else T - 1 - t
                        xwt = xwp.tile([RW, G4], F16, tag=f"xwt{d}", name=f"xwt{d}")
                        nc.sync.dma_start(xwt[:], xw[d][:, tx, :])
                        sgall = gp.tile([RW, G4], F32, tag=f"sgall{d}",
                                        name=f"sgall{d}")

                        def bank(nj):
                            return slice(nj * H, (nj + 1) * H)

                        if t == 0:
                            # h == 0: gates are just xw + bias (bias folded in xw)
                            for nj in range(4):
                                nc.vector.tensor_copy(sgall[:, bank(nj)],
                                                      xwt[:, bank(nj)])
                        else:
                            gps = gps_pool.tile([RW, G4], F32, tag=f"g{d}",
                                                name=f"gps{d}")
                            for nj in range(4):
                                for kc in range(4):
                                    nc.tensor.matmul(
                                        gps[:, bank(nj)],
                                        hT[d][:, 32 * kc:32 * kc + RW],
                                        whh_sb[d][kc][:, bank(nj)],
                                        start=(kc == 0), stop=(kc == 3))
                                nc.vector.tensor_add(sgall[:, bank(nj)],
                                                     gps[:, bank(nj)],
                                                     xwt[:, bank(nj)])
                        # activations in-place per gate; order [g, i, f, o]
                        nc.scalar.activation(sgall[:, bank(GG)], sgall[:, bank(GG)],
                                             AF.Tanh)
                        nc.scalar.activation(sgall[:, bank(GI)], sgall[:, bank(GI)],
                                             AF.Sigmoid)
                        p_ = gp.tile([RW, H], F32, tag=f"p_{d}", name=f"p_{d}")
                        nc.gpsimd.tensor_mul(p_[:], sgall[:, bank(GI)],
                                             sgall[:, bank(GG)])
                        nc.scalar.activation(sgall[:, bank(GF)], sgall[:, bank(GF)],
                                             AF.Sigmoid)
                        if t == 0:
                            nc.vector.tensor_copy(c_st[d][:], p_[:])
                        else:
                            q_ = gp.tile([RW, H], F32, tag=f"q_{d}", name=f"q_{d}")
                            nc.gpsimd.tensor_mul(q_[:], sgall[:, bank(GF)], c_st[d][:])
                            nc.vector.tensor_add(c_st[d][:], p_[:], q_[:])
                        nc.scalar.activation(sgall[:, bank(GO)], sgall[:, bank(GO)],
                                             AF.Sigmoid)
                        th = gp.tile([RW, H], F32, tag=f"th{d}", name=f"th{d}")
                        nc.scalar.activation(th[:], c_st[d][:], AF.Tanh)
                        h_ = gp.tile([RW, H], F32R, tag=f"h_{d}", name=f"h_{d}")
                        nc.vector.tensor_mul(h_[:], sgall[:, bank(GO)], th[:])
                        tp = gps_pool.tile([128, 4 * RW], F32R, tag=f"g{d}",
                                           name=f"tp{d}")
                        for cc in range(4):
                            nc.tensor.transpose(tp[:, RW * cc:RW * cc + RW],
                                                h_[:, 128 * cc:128 * cc + 128],
                                                identr[0:RW, 0:RW])
                        nc.vector.tensor_copy(hT[d][:], tp[:])
                        lo, hi = (0, H) if d == "f" else (H, H2)
                        nc.sync.dma_start(tm[:, tx, lo:hi], h_[:])
                        nc.sync.dma_start(outA_d.ap()[:, tx, lo:hi],
                                          h_[0:PB, :].bitcast(F32))
                        nc.sync.dma_start(outB_d.ap()[:, tx, lo:hi],
                                          h_[PB:RW, :].bitcast(F32))

            # ---------------- Phase 3: attention + enhancement ----------------
            if phases < 3:
                nc.compile()
                return nc
            with tc.tile_pool(name="a3", bufs=a3_bufs) as a3, \
                 tc.tile_pool(name="a3s", bufs=2) as a3s, \
                 tc.tile_pool(name="eps", bufs=eps_bufs, space="PSUM") as eps_pool, \
                 tc.tile_pool(name="tp3", bufs=3, space="PSUM") as tp3_pool, \
                 tc.tile_pool(name="ops", bufs=1, space="PSUM") as ops_pool:
                for n in range(PB):
                    a_tm = a3.tile([128, H2], F32R, tag="a_tm")
                    nc.sync.dma_start(a_tm[:], tm[n, :, :])
                    b_tm = a3.tile([128, H2], F32R, tag="b_tm")
                    nc.sync.dma_start(b_tm[:], tm[PB + n, :, :])
                    a_fm = a3.tile([128, H2], F32, tag="a_fm")
                    b_fm = a3.tile([128, H2], F32, tag="b_fm")
                    for src, dst in ((a_tm, a_fm), (b_tm, b_fm)):
                        for cc in range(8):
                            tp3 = tp3_pool.tile([128, 128], F32R, tag="tp3")
                            nc.tensor.transpose(tp3[:], src[:, 128 * cc:128 * (cc + 1)],
                                                identr[:])
                            nc.vector.tensor_copy(dst[:, 128 * cc:128 * (cc + 1)],
                                                  tp3[:].bitcast(F32))
                    e_ps = eps_pool.tile([128, 128], F32, tag="e")
                    e2_ps = eps_pool.tile([128, 128], F32, tag="e")
                    for cc in range(8):
                        sl = slice(128 * cc, 128 * (cc + 1))
                        nc.tensor.matmul(e_ps[:], a_fm[:, sl], b_fm[:, sl],
                                         start=(cc == 0), stop=(cc == 7))
                    for cc in range(8):
                        sl = slice(128 * cc, 128 * (cc + 1))
                        nc.tensor.matmul(e2_ps[:], b_fm[:, sl], a_fm[:, sl],
                                         start=(cc == 0), stop=(cc == 7))
                    zs, rs = [], []
                    for eps in (e_ps, e2_ps):
                        m_ = a3s.tile([128, 1], F32, tag="m_")
                        nc.vector.tensor_reduce(m_[:], eps[:], axis=AX.X,
                                                op=ALU.max, negate=True)
                        z_ = a3s.tile([128, 128], F32, tag="z_")
                        s_ = a3s.tile([128, 1], F32, tag="s_")
                        nc.scalar.activation(z_[:], eps[:], AF.Exp, bias=m_[:],
                                             accum_out=s_[:])
                        r_ = a3s.tile([128, 1], F32, tag="r_")
                        nc.vector.reciprocal(r_[:], s_[:])
                        zt_ps = tp3_pool.tile([128, 128], F32, tag="tp3")
                        nc.tensor.transpose(zt_ps[:], z_[:], ident[:])
                        zt = a3s.tile([128, 128], F32R, tag="zt")
                        nc.vector.tensor_copy(zt[:], zt_ps[:])
                        zs.append(zt)
                        rs.append(r_)
                    tilded = []
                    for zt, r_, rhs_tm in ((zs[0], rs[0], b_tm), (zs[1], rs[1], a_tm)):
                        t_ps = ops_pool.tile([128, H2], F32, tag="t_ps")
                        for half in range(2):
                            sl = slice(512 * half, 512 * (half + 1))
                            nc.tensor.matmul(t_ps[:, sl], zt[:], rhs_tm[:, sl],
                                             start=True, stop=True)
                        # assemble [til | diff | prod] contiguously, one DMA out
                        big = a3.tile([128, 3 * H2], F32, tag="big")
                        nc.vector.tensor_scalar_mul(big[:, 0:H2], t_ps[:], r_[:])
                        tilded.append(big)
                    for bar, big, outd in ((a_tm, tilded[0], outA_d),
                                           (b_tm, tilded[1], outB_d)):
                        nc.gpsimd.tensor_sub(big[:, H2:2 * H2], bar[:].bitcast(F32),
                                             big[:, 0:H2])
                        nc.vector.tensor_mul(big[:, 2 * H2:3 * H2],
                                             bar[:].bitcast(F32), big[:, 0:H2])
                        nc.sync.dma_start(outd.ap()[n, :, H2:4 * H2], big[:])

    nc.compile()
    return nc


def _get_nc():
    if "nc" not in _CACHE:
        _CACHE["nc"] = _build()
    return _CACHE["nc"]


def prep_in_maps(inputs):
    A = np.asarray(inputs["A"])
    B = np.asarray(inputs["B"])
    embed = np.asarray(inputs["embed"], dtype=np.float32)
    # permute pytorch gate order [i,f,g,o] -> [g,i,f,o]
    perm = np.concatenate([np.arange(2 * H, 3 * H), np.arange(0, 2 * H),
                           np.arange(3 * H, 4 * H)])
    wmat, bmat = {}, {}
    for d in "fb":
        suf = "_f" if d == "f" else "_b"
        wihT = np.ascontiguousarray(
            np.asarray(inputs["Wih" + suf], dtype=np.float32)[perm].T)
        whhT = np.ascontiguousarray(
            np.asarray(inputs["Whh" + suf], dtype=np.float32)[perm].T)
        bias = (np.asarray(inputs["bih" + suf], dtype=np.float32)
                + np.asarray(inputs["bhh" + suf], dtype=np.float32))[perm]
        bias_bc = np.ascontiguousarray(
            np.broadcast_to(bias[None, :], (128, G4)), dtype=np.float32)
        wmat[d] = (wihT, whhT)
        bmat[d] = bias_bc

    xa = embed[A]    # [BSZ, T, E]
    xb = embed[B]

    in_maps = []
    for c in range(NCORES):
        sl = slice(PB * c, PB * (c + 1))
        xc = np.concatenate([xa[sl], xb[sl]], axis=0)          # [RW, T, E]
        xT = np.ascontiguousarray(
            xc.transpose(2, 0, 1).reshape(E, RW * T), dtype=np.float32)
        in_maps.append({
            "xT": xT,
            "wihT_f": wmat["f"][0], "whhT_f": wmat["f"][1], "bias_f": bmat["f"],
            "wihT_b": wmat["b"][0], "whhT_b": wmat["b"][1], "bias_b": bmat["b"],
        })
    return in_maps


def kernel(**inputs):
    from concourse.bass_utils import run_bass_kernel_spmd

    in_maps = prep_in_maps(inputs)
    nc = _get_nc()
    res = run_bass_kernel_spmd(nc, in_maps, core_ids=list(range(NCORES)))
    outA = np.concatenate([res.results[c]["outA"] for c in range(NCORES)], axis=0)
    outB = np.concatenate([res.results[c]["outB"] for c in range(NCORES)], axis=0)
    return outA, outB


# ---------------------------------------------------------------------------
# Two-NEFF variant: run1 = proj + one (seq, dir, half-batch) scan per core;
# run2 = batch-sharded attention. Host reshuffles hidden states in between and
# writes the "bar" output quarter directly from run1's results.
B1 = 64  # batch rows per run1 core


def _build_run1():
    import concourse.mybir as mybir
    import concourse.tile as tile
    from concourse import bacc
    from concourse.masks import make_identity

    F32 = mybir.dt.float32
    F32R = mybir.dt.float32r
    F16 = mybir.dt.float16
    AF = mybir.ActivationFunctionType

    nc = bacc.Bacc("TRN2", target_bir_lowering=False, debug=False,
                   num_devices=NCORES)
    xT_d = nc.dram_tensor("xT", [E, B1 * T], F32R, kind="ExternalInput")
    wih_d = nc.dram_tensor("wihT", [E, G4], F32R, kind="ExternalInput")
    whh_d = nc.dram_tensor("whhT", [H, G4], F32R, kind="ExternalInput")
    bias_d = nc.dram_tensor("bias", [128, G4], F32, kind="ExternalInput")
    tm_d = nc.dram_tensor("tm1", [B1, T, H], F32, kind="ExternalOutput")

    with tile.TileContext(nc) as tc:
        with tc.tile_pool(name="dram", bufs=1, space="DRAM") as dpool, \
             tc.tile_pool(name="const", bufs=1) as const:
            xw = dpool.tile([B1, T, G4], F16, name="xw1")
            ident = const.tile([128, 128], F32)
            make_identity(nc, ident[:])
            identr = const.tile([128, 128], F32R)
            nc.vector.tensor_copy(identr[:], ident[:])

            # proj
            with tc.tile_pool(name="p1w", bufs=1) as p1w, \
                 tc.tile_pool(name="p1ps", bufs=2, space="PSUM") as p1ps, \
                 tc.tile_pool(name="p1e", bufs=3) as p1e:
                xT_sb, wih_sb = [], []
                for ki, (ko, ks) in enumerate(KCH):
                    t_ = p1w.tile([ks, B1 * T], F32R, tag=f"xT{ki}", name=f"xT{ki}")
                    nc.sync.dma_start(t_[:], xT_d.ap()[ko:ko + ks, :])
                    xT_sb.append(t_)
                    w_ = p1w.tile([ks, G4], F32R, tag=f"wih{ki}", name=f"wih{ki}")
                    nc.sync.dma_start(w_[:], wih_d.ap()[ko:ko + ks, :])
                    wih_sb.append(w_)
                bias_sb = p1w.tile([128, G4], F32, tag="bias")
                nc.sync.dma_start(bias_sb[:], bias_d.ap())
                for rc in range(B1):
                    ps = p1ps.tile([128, G4], F32, tag="pj")
                    for nj in range(4):
                        for ki in range(3):
                            nc.tensor.matmul(
                                ps[:, nj * 512:(nj + 1) * 512],
                                xT_sb[ki][:, rc * T:(rc + 1) * T],
                                wih_sb[ki][:, nj * 512:(nj + 1) * 512],
                                start=(ki == 0), stop=(ki == 2))
                    ev = p1e.tile([128, G4], F16, tag="ev")
                    nc.vector.tensor_add(ev[:], ps[:], bias_sb[:])
                    nc.sync.dma_start(xw[rc, :, :], ev[:])

            # scan (single direction; bwd cores get host-reversed inputs)
            with tc.tile_pool(name="wst", bufs=1) as wst, \
                 tc.tile_pool(name="sst", bufs=1) as sst, \
                 tc.tile_pool(name="xwp", bufs=3) as xwp, \
                 tc.tile_pool(name="gp", bufs=2) as gp, \
                 tc.tile_pool(name="gps", bufs=1, space="PSUM") as gps_pool:
                whh_sb = []
                for kc in range(4):
                    w = wst.tile([128, G4], F32R, tag=f"whh{kc}", name=f"whh{kc}")
                    nc.sync.dma_start(w[:], whh_d.ap()[kc * 128:(kc + 1) * 128, :])
                    whh_sb.append(w)
                hT = sst.tile([128, 4 * B1], F32R, name="hT1")
                c_st = sst.tile([B1, H], F32, name="c_st1")

                GG, GI, GF, GO = 0, 1, 2, 3
                for t in range(T):
                    xwt = xwp.tile([B1, G4], F16, tag="xwt", name="xwt")
                    nc.sync.dma_start(xwt[:], xw[:, t, :])
                    sgall = gp.tile([B1, G4], F32, tag="sgall", name="sgall")

                    def bank(nj):
                        return slice(nj * H, (nj + 1) * H)

                    if t == 0:
                        for nj in range(4):
                            nc.vector.tensor_copy(sgall[:, bank(nj)],
                                                  xwt[:, bank(nj)])
                    else:
                        gps = gps_pool.tile([B1, G4], F32, tag="g", name="gps1")
                        for nj in range(4):
                            for kc in range(4):
                                nc.tensor.matmul(
                                    gps[:, bank(nj)],
                                    hT[:, B1 * kc:B1 * kc + B1],
                                    whh_sb[kc][:, bank(nj)],
                                    start=(kc == 0), stop=(kc == 3))
                            nc.vector.tensor_add(sgall[:, bank(nj)],
                                                 gps[:, bank(nj)],
                                                 xwt[:, bank(nj)])
                    nc.scalar.activation(sgall[:, bank(GG)], sgall[:, bank(GG)],
                                         AF.Tanh)
                    nc.scalar.activation(sgall[:, bank(GI)], sgall[:, bank(GI)],
                                         AF.Sigmoid)
                    p_ = gp.tile([B1, H], F32, tag="p_", name="p_")
                    nc.gpsimd.tensor_mul(p_[:], sgall[:, bank(GI)],
                                         sgall[:, bank(GG)])
                    nc.scalar.activation(sgall[:, bank(GF)], sgall[:, bank(GF)],
                                         AF.Sigmoid)
                    if t == 0:
                        nc.vector.tensor_copy(c_st[:], p_[:])
                    else:
                        q_ = gp.tile([B1, H], F32, tag="q_", name="q_")
                        nc.gpsimd.tensor_mul(q_[:], sgall[:, bank(GF)], c_st[:])
                        nc.vector.tensor_add(c_st[:], p_[:], q_[:])
                    nc.scalar.activation(sgall[:, bank(GO)], sgall[:, bank(GO)],
                                         AF.Sigmoid)
                    th = gp.tile([B1, H], F32, tag="th", name="th")
                    nc.scalar.activation(th[:], c_st[:], AF.Tanh)
                    h_ = gp.tile([B1, H], F32, tag="h_", name="h_")
                    nc.vector.tensor_mul(h_[:], sgall[:, bank(GO)], th[:])
                    tp = gps_pool.tile([128, 4 * B1], F32R, tag="g", name="tp1")
                    for cc in range(4):
                        nc.tensor.transpose(tp[:, B1 * cc:B1 * cc + B1],
                                            h_[:, 128 * cc:128 * cc + 128]
                                            .bitcast(F32R),
                                            identr[0:B1, 0:B1])
                    nc.vector.tensor_copy(hT[:], tp[:])
                    nc.sync.dma_start(tm_d.ap()[:, t, :], h_[:])
    nc.compile()
    return nc


def _build_run2():
    import concourse.mybir as mybir
    import concourse.tile as tile
    from concourse import bacc
    from concourse.masks import make_identity

    F32 = mybir.dt.float32
    F32R = mybir.dt.float32r
    AF = mybir.ActivationFunctionType
    ALU = mybir.AluOpType
    AX = mybir.AxisListType

    nc = bacc.Bacc("TRN2", target_bir_lowering=False, debug=False,
                   num_devices=NCORES)
    tmA_d = nc.dram_tensor("tmA", [PB, T, H2], F32R, kind="ExternalInput")
    tmB_d = nc.dram_tensor("tmB", [PB, T, H2], F32R, kind="ExternalInput")
    oA_d = nc.dram_tensor("oA", [PB, T, 3 * H2], F32, kind="ExternalOutput")
    oB_d = nc.dram_tensor("oB", [PB, T, 3 * H2], F32, kind="ExternalOutput")

    with tile.TileContext(nc) as tc:
        with tc.tile_pool(name="const", bufs=1) as const, \
             tc.tile_pool(name="a3", bufs=2) as a3, \
             tc.tile_pool(name="a3s", bufs=2) as a3s, \
             tc.tile_pool(name="eps", bufs=2, space="PSUM") as eps_pool, \
             tc.tile_pool(name="tp3", bufs=3, space="PSUM") as tp3_pool, \
             tc.tile_pool(name="ops", bufs=1, space="PSUM") as ops_pool:
            ident = const.tile([128, 128], F32)
            make_identity(nc, ident[:])
            identr = const.tile([128, 128], F32R)
            nc.vector.tensor_copy(identr[:], ident[:])
            for n in range(PB):
                a_tm = a3.tile([128, H2], F32R, tag="a_tm")
                nc.sync.dma_start(a_tm[:], tmA_d.ap()[n, :, :])
                b_tm = a3.tile([128, H2], F32R, tag="b_tm")
                nc.sync.dma_start(b_tm[:], tmB_d.ap()[n, :, :])
                a_fm = a3.tile([128, H2], F32, tag="a_fm")
                b_fm = a3.tile([128, H2], F32, tag="b_fm")
                for src_, dst in ((a_tm, a_fm), (b_tm, b_fm)):
                    for cc in range(8):
                        tp3 = tp3_pool.tile([128, 128], F32R, tag="tp3")
                        nc.tensor.transpose(tp3[:], src_[:, 128 * cc:128 * (cc + 1)],
                                            identr[:])
                        nc.vector.tensor_copy(dst[:, 128 * cc:128 * (cc + 1)],
                                              tp3[:].bitcast(F32))
                e_ps = eps_pool.tile([128, 128], F32, tag="e")
                e2_ps = eps_pool.tile([128, 128], F32, tag="e")
                for cc in range(8):
                    sl = slice(128 * cc, 128 * (cc + 1))
                    nc.tensor.matmul(e_ps[:], a_fm[:, sl], b_fm[:, sl],
                                     start=(cc == 0), stop=(cc == 7))
                for cc in range(8):
                    sl = slice(128 * cc, 128 * (cc + 1))
                    nc.tensor.matmul(e2_ps[:], b_fm[:, sl], a_fm[:, sl],
                                     start=(cc == 0), stop=(cc == 7))
                zs, rs = [], []
                for eps in (e_ps, e2_ps):
                    m_ = a3s.tile([128, 1], F32, tag="m_")
                    nc.vector.tensor_reduce(m_[:], eps[:], axis=AX.X,
                                            op=ALU.max, negate=True)
                    z_ = a3s.tile([128, 128], F32, tag="z_")
                    s_ = a3s.tile([128, 1], F32, tag="s_")
                    nc.scalar.activation(z_[:], eps[:], AF.Exp, bias=m_[:],
                                         accum_out=s_[:])
                    r_ = a3s.tile([128, 1], F32, tag="r_")
                    nc.vector.reciprocal(r_[:], s_[:])
                    zt_ps = tp3_pool.tile([128, 128], F32, tag="tp3")
                    nc.tensor.transpose(zt_ps[:], z_[:], ident[:])
                    zt = a3s.tile([128, 128], F32R, tag="zt")
                    nc.vector.tensor_copy(zt[:], zt_ps[:])
                    zs.append(zt)
                    rs.append(r_)
                tilded = []
                for zt, r_, rhs_tm in ((zs[0], rs[0], b_tm), (zs[1], rs[1], a_tm)):
                    t_ps = ops_pool.tile([128, H2], F32, tag="t_ps")
                    for half in range(2):
                        sl = slice(512 * half, 512 * (half + 1))
                        nc.tensor.matmul(t_ps[:, sl], zt[:], rhs_tm[:, sl],
                                         start=True, stop=True)
                    til = a3.tile([128, H2], F32, tag="til")
                    nc.vector.tensor_scalar_mul(til[:], t_ps[:], r_[:])
                    tilded.append(til)
                for bar, til, outd in ((a_tm, tilded[0], oA_d),
                                       (b_tm, tilded[1], oB_d)):
                    nc.sync.dma_start(outd.ap()[n, :, 0:H2], til[:])
                    df = a3.tile([128, H2], F32, tag="df")
                    nc.gpsimd.tensor_sub(df[:], bar[:].bitcast(F32), til[:])
                    nc.sync.dma_start(outd.ap()[n, :, H2:2 * H2], df[:])
                    pr = a3.tile([128, H2], F32, tag="pr")
                    nc.vector.tensor_mul(pr[:], bar[:].bitcast(F32), til[:])
                    nc.sync.dma_start(outd.ap()[n, :, 2 * H2:3 * H2], pr[:])
    nc.compile()
    return nc


def kernel2(**inputs):
    """Two-NEFF variant: run1 scans, host reshuffle, run2 attention."""
    from concourse.bass_utils import run_bass_kernel_spmd

    A = np.asarray(inputs["A"])
    B = np.asarray(inputs["B"])
    embed = np.asarray(inputs["embed"], dtype=np.float32)
    perm = np.concatenate([np.arange(2 * H, 3 * H), np.arange(0, 2 * H),
                           np.arange(3 * H, 4 * H)])
    wp = {}
    for d in "fb":
        suf = "_f" if d == "f" else "_b"
        wihT = np.ascontiguousarray(
            np.asarray(inputs["Wih" + suf], dtype=np.float32)[perm].T)
        whhT = np.ascontiguousarray(
            np.asarray(inputs["Whh" + suf], dtype=np.float32)[perm].T)
        bias = (np.asarray(inputs["bih" + suf], dtype=np.float32)
                + np.asarray(inputs["bhh" + suf], dtype=np.float32))[perm]
        bias_bc = np.ascontiguousarray(
            np.broadcast_to(bias[None, :], (128, G4)), dtype=np.float32)
        wp[d] = (wihT, whhT, bias_bc)

    x_seq = {0: embed[A], 1: embed[B]}   # [BSZ, T, E]

    # core c: seq = c//4, dir = (c//2)%2 (0=f,1=b), half = c%2
    in_maps1 = []
    meta = []
    for c in range(NCORES):
        seq, dirb, half = c // 4, (c // 2) % 2, c % 2
        d = "fb"[dirb]
        xs = x_seq[seq][B1 * half:B1 * (half + 1)]       # [64, T, E]
        if d == "b":
            xs = xs[:, ::-1, :]                          # reversed time
        xT = np.ascontiguousarray(
            xs.transpose(2, 0, 1).reshape(E, B1 * T), dtype=np.float32)
        wihT, whhT, bias_bc = wp[d]
        in_maps1.append({"xT": xT, "wihT": wihT, "whhT": whhT, "bias": bias_bc})
        meta.append((seq, d, half))

    if "nc1" not in _CACHE:
        _CACHE["nc1"] = _build_run1()
    res1 = run_bass_kernel_spmd(_CACHE["nc1"], in_maps1,
                                core_ids=list(range(NCORES)))

    tm_full = {0: np.empty((BSZ, T, H2), np.float32),
               1: np.empty((BSZ, T, H2), np.float32)}
    for c, (seq, d, half) in enumerate(meta):
        tm1 = res1.results[c]["tm1"]                     # [64, T, 512]
        if d == "b":
            tm1 = tm1[:, ::-1, :]
        lo = 0 if d == "f" else H
        tm_full[seq][B1 * half:B1 * (half + 1), :, lo:lo + H] = tm1

    in_maps2 = []
    for c in range(NCORES):
        sl = slice(PB * c, PB * (c + 1))
        in_maps2.append({
            "tmA": np.ascontiguousarray(tm_full[0][sl]),
            "tmB": np.ascontiguousarray(tm_full[1][sl]),
        })
    if "nc2" not in _CACHE:
        _CACHE["nc2"] = _build_run2()
    res2 = run_bass_kernel_spmd(_CACHE["nc2"], in_maps2,
                                core_ids=list(range(NCORES)))

    outA = np.empty((BSZ, T, 4 * H2), np.float32)
    outB = np.empty((BSZ, T, 4 * H2), np.float32)
    outA[:, :, 0:H2] = tm_full[0]
    outB[:, :, 0:H2] = tm_full[1]
    for c in range(NCORES):
        sl = slice(PB * c, PB * (c + 1))
        outA[sl, :, H2:] = res2.results[c]["oA"]
        outB[sl, :, H2:] = res2.results[c]["oB"]
    return outA, outB



# revision 31
# speedup vs baseline: 3.2554x; 3.2554x over previous
"""Bass/Trainium2 kernel for nn_Encoder (embedding -> BiLSTM -> cross attention
-> enhancement).

Sharding: data-parallel over batch, 16 items per core on 8 NeuronCores (no
collectives). Per core the A and B sequences are stacked into 32 batch rows.

Layout: everything transposed vs the torch reference. Gates are computed as
g^T[2048, 32] = W_ih @ x_t^T + W_hh @ h^T with the gate dimension on SBUF
partitions and the 32 batch rows on the matmul moving dim, so every matmul
streams fp16 at 1 cycle/row and every vector/scalar op runs on all 128
partitions. The x-projection is inlined into the scan (3 extra fp16 K-chunks
per gate chunk; bias rides along as a ones-row of x), and is emitted 2-3 steps
ahead of the recurrent matmuls to keep the PE array continuously busy (full
p-state). h_t is written by the DVE directly into a persistent fp16 history
tile [128, T, 4, 32] per direction, which then serves as (a) the next step's
matmul rhs, (b) the feature-major attention operands via strided APs, and
(c) the source for the time-major tiles via PE transposes. Outputs are
assembled per item in one [128, 4096] fp16 tile per side ([bar|til|diff|prod])
and leave in a single DMA; the host widens to f32.
"""

import numpy as np

V, E, H = 32000, 300, 512
BSZ, T = 128, 128
NCORES = 8
PB = BSZ // NCORES          # 16 batch items per core per sequence
RW = 2 * PB                 # 32 stacked rows (A items then B items)
G4 = 4 * H                  # 2048 gate width
H2 = 2 * H                  # 1024 bilstm output width
EA = 384                    # padded x feature dim (300 + bias row + zeros)
XC = 3                      # x K-chunks of 128
HC = 4                      # h K-chunks of 128
XAHEAD = 3                  # steps of x-projection prefill ahead of the scan

_CACHE = {}


def _build():
    import concourse.mybir as mybir
    import concourse.tile as tile
    from concourse import bacc
    from concourse.masks import make_identity

    F32 = mybir.dt.float32
    F16 = mybir.dt.float16
    AF = mybir.ActivationFunctionType
    ALU = mybir.AluOpType
    AX = mybir.AxisListType

    nc = bacc.Bacc("TRN2", target_bir_lowering=False, debug=False,
                   num_devices=NCORES)

    # Host-prepped inputs (see prep_in_maps): x^T with ones row for the bias,
    # and per-direction weights stacked as 7 K-chunks (3 wih+bias, 4 whh).
    xT_d = nc.dram_tensor("xT", [EA, RW * T], F16, kind="ExternalInput")
    w_d = {d: nc.dram_tensor(f"w_{d}", [7 * 128, G4], F16, kind="ExternalInput")
           for d in "fb"}
    # device ships [bar | til] per item; host derives bar-til and bar*til
    outA_d = nc.dram_tensor("outA", [PB, T, 2 * H2], F16, kind="ExternalOutput")
    outB_d = nc.dram_tensor("outB", [PB, T, 2 * H2], F16, kind="ExternalOutput")

    with tile.TileContext(nc) as tc:
        with tc.tile_pool(name="const", bufs=1) as const, \
             tc.tile_pool(name="hist", bufs=1) as histp:
            ident = const.tile([128, 128], F32)
            make_identity(nc, ident[:])
            ident16 = const.tile([128, 128], F16)
            nc.vector.tensor_copy(ident16[:], ident[:])

            # persistent fp16 h history, [128 d-in-chunk, T, h-chunk, batch]
            hist = {d: histp.tile([128, T, HC, RW], F16, name=f"hist_{d}")
                    for d in "fb"}

            # ---------------- Phase 1+2: fused projection + scan ----------
            with tc.tile_pool(name="wst", bufs=1) as wst, \
                 tc.tile_pool(name="xst", bufs=1) as xst, \
                 tc.tile_pool(name="cst", bufs=1) as cst, \
                 tc.tile_pool(name="gct", bufs=2) as gct, \
                 tc.tile_pool(name="ew", bufs=2) as ew, \
                 tc.tile_pool(name="gps", bufs=XAHEAD + 1, space="PSUM") as gps:
                wsb = {}
                for d in "fb":
                    wsb[d] = wst.tile([128, 7, G4], F16, name=f"w_{d}")
                    nc.sync.dma_start(
                        wsb[d][:],
                        w_d[d].ap().rearrange("(k p) c -> p k c", p=128))
                xk = xst.tile([128, XC, RW, T], F16, name="xk")
                nc.sync.dma_start(
                    xk[:], xT_d.ap().rearrange("(k p) c -> p k c", p=128))
                c_st = {d: cst.tile([128, HC, RW], F16, name=f"c_{d}")
                        for d in "fb"}

                ps_tiles = {}

                # One accumulation group per PSUM bank per step: start=True on
                # the bank's first matmul marks the whole 2KB zero region, so
                # every gate chunk's first write lands zeroed; stop=True on
                # the bank's final matmul.  This keeps the x-projection
                # prefill (emitted XAHEAD steps early) in the same group as
                # the recurrent matmuls without interleaved-group violations.
                def emit_x(d, s):
                    tx = s if d == "f" else T - 1 - s
                    ps = gps.tile([128, 16, RW], F32, tag=f"ps{d}",
                                  name=f"ps{d}")
                    ps_tiles[(d, s)] = ps
                    last = s == 0
                    for gc in range(16):
                        for kx in range(XC):
                            nc.tensor.matmul(
                                ps[:, gc, :],
                                wsb[d][:, kx, gc * 128:(gc + 1) * 128],
                                xk[:, kx, :, tx],
                                start=(gc == 0 and kx == 0),
                                stop=(last and gc == 15 and kx == XC - 1))

                def emit_h(d, s):
                    tprev = (s - 1) if d == "f" else (T - s)
                    ps = ps_tiles[(d, s)]
                    hp = hist[d][:, tprev, :, :]
                    for gc in range(16):
                        for kc in range(HC):
                            nc.tensor.matmul(
                                ps[:, gc, :],
                                wsb[d][:, XC + kc, gc * 128:(gc + 1) * 128],
                                hp[:, kc, :],
                                start=False,
                                stop=(gc == 15 and kc == HC - 1))

                # Gate chunk layout (host-permuted): [f | i | g | o], 4 chunks
                # of 128 gate dims each.
                for s in range(min(XAHEAD, T)):
                    emit_x("f", s)
                    emit_x("b", s)
                for s in range(T):
                    da, db = ("f", "b") if s % 2 == 0 else ("b", "f")
                    if s >= 1:
                        emit_h(da, s)
                        emit_h(db, s)
                    if s + XAHEAD < T:
                        emit_x(da, s + XAHEAD)
                        emit_x(db, s + XAHEAD)
                    # Activation-queue order tuned so the f-chain's tanh(c)
                    # is not stuck behind the b-dir gate activations, and the
                    # b-chain exploits its natural slack (its matmuls sit in
                    # the second half of each PE burst).
                    ga, th_t = {}, {}
                    for d in "fb":
                        ps = ps_tiles.pop((d, s))
                        g = gct.tile([128, 16, RW], F16, tag=f"ga{d}",
                                     name=f"ga{d}")
                        ga[d] = (g, ps)
                        th_t[d] = ew.tile([128, HC, RW], F16, tag=f"th{d}",
                                          name=f"th{d}")

                    def act_sfi(d):
                        g, ps = ga[d]
                        nc.scalar.activation(g[:, 0:8, :], ps[:, 0:8, :],
                                             AF.Sigmoid)          # f, i

                    def act_tg(d):
                        g, ps = ga[d]
                        nc.scalar.activation(g[:, 8:12, :], ps[:, 8:12, :],
                                             AF.Tanh)             # g

                    def act_so(d):
                        g, ps = ga[d]
                        nc.scalar.activation(g[:, 12:16, :], ps[:, 12:16, :],
                                             AF.Sigmoid)          # o

                    def act_tc(d):
                        nc.scalar.activation(th_t[d][:], c_st[d][:], AF.Tanh)

                    def dve_q(d):
                        if s == 0:
                            return None
                        g, _ = ga[d]
                        q_ = ew.tile([128, HC, RW], F16, tag=f"q{d}",
                                     name=f"q{d}")
                        nc.vector.tensor_mul(q_[:], g[:, 0:4, :], c_st[d][:])
                        return q_

                    def dve_pc(d, q_):
                        g, _ = ga[d]
                        if s == 0:
                            nc.vector.tensor_mul(c_st[d][:], g[:, 4:8, :],
                                                 g[:, 8:12, :])   # c = i*g
                        else:
                            p_ = ew.tile([128, HC, RW], F16, tag=f"p{d}",
                                         name=f"p{d}")
                            nc.vector.tensor_mul(p_[:], g[:, 4:8, :],
                                                 g[:, 8:12, :])   # i*g
                            nc.vector.tensor_add(c_st[d][:], p_[:], q_[:])

                    def dve_h(d):
                        tx = s if d == "f" else T - 1 - s
                        nc.vector.tensor_mul(hist[d][:, tx, :, :],
                                             ga[d][0][:, 12:16, :], th_t[d][:])

                    # Alternate which direction leads each step so the act
                    # queue penalty of going second is shared evenly.
                    d1, d2 = ("f", "b") if s % 2 == 0 else ("b", "f")
                    act_sfi(d1)
                    act_tg(d1)
                    act_so(d1)
                    q1 = dve_q(d1)
                    dve_pc(d1, q1)
                    act_sfi(d2)
                    act_tc(d1)
                    q2 = dve_q(d2)
                    dve_h(d1)
                    act_tg(d2)
                    act_so(d2)
                    dve_pc(d2, q2)
                    act_tc(d2)
                    dve_h(d2)

            # ---------------- Phase 3: attention + enhancement -------------
            with tc.tile_pool(name="big", bufs=5) as bigp, \
                 tc.tile_pool(name="sm", bufs=6) as smp, \
                 tc.tile_pool(name="zt", bufs=2) as ztp, \
                 tc.tile_pool(name="eps", bufs=2, space="PSUM") as eps_pool, \
                 tc.tile_pool(name="tp", bufs=4, space="PSUM") as tp_pool, \
                 tc.tile_pool(name="tps", bufs=1, space="PSUM") as tps_pool:

                def fm(side, c8, n):
                    # feature-major [128 d, 128 t] strided view of history
                    d = "f" if c8 < 4 else "b"
                    return hist[d][:, :, c8 % 4, n if side == 0 else PB + n]

                def stage_a(n):
                    # E matmuls straight off the history (strided APs);
                    # e1/e2 pack into one PSUM bank
                    e12 = eps_pool.tile([128, 2, 128], F32, tag="e", name="e12")
                    e1, e2 = e12[:, 0, :], e12[:, 1, :]
                    for c8 in range(8):
                        nc.tensor.matmul(e1, fm(0, c8, n), fm(1, c8, n),
                                         start=(c8 == 0), stop=(c8 == 7))
                    for c8 in range(8):
                        nc.tensor.matmul(e2, fm(1, c8, n), fm(0, c8, n),
                                         start=(c8 == 0), stop=(c8 == 7))

                    # time-major tiles: 4 PE transposes batched per PSUM bank
                    # (one pending-zero group), then a single wide copy per
                    # bank alternating Act/DVE (GPSIMD cannot read PSUM).
                    # big layout: [bar | til]
                    big = [bigp.tile([128, 2 * H2], F16, tag=f"big{sd}",
                                     name=f"big{sd}") for sd in range(2)]
                    k = 0
                    for sd in range(2):
                        for half in range(2):
                            tp = tp_pool.tile([128, 4, 128], F16, tag="tp",
                                              name="tp")
                            for j in range(4):
                                nc.tensor.matmul(
                                    tp[:, j, :], fm(sd, half * 4 + j, n),
                                    ident16[:], is_transpose=True,
                                    start=(j == 0), stop=(j == 3))
                            dst = big[sd][:, half * 512:(half + 1) * 512]
                            if k % 2 == 0:
                                nc.scalar.activation(dst, tp[:, :, :], AF.Copy)
                            else:
                                nc.vector.tensor_copy(dst, tp[:, :, :])
                            k += 1

                    # softmaxes (row softmax of E and of E^T)
                    zs, rs = [], []
                    for eps in (e1, e2):
                        m_ = smp.tile([128, 1], F32, tag="m")
                        nc.vector.tensor_reduce(m_[:], eps[:], axis=AX.X,
                                                op=ALU.max, negate=True)
                        z_ = smp.tile([128, 128], F16, tag="z")
                        s_ = smp.tile([128, 1], F32, tag="s")
                        nc.scalar.activation(z_[:], eps[:], AF.Exp, bias=m_[:],
                                             accum_out=s_[:])
                        r_ = smp.tile([128, 1], F32, tag="r")
                        nc.vector.reciprocal(r_[:], s_[:])
                        zs.append(z_)
                        rs.append(r_)
                    return zs, rs, big

                def stage_b(n, zs, rs, big):
                    # z transposes + align matmuls + enhancement + out DMA
                    for sd, (z_, othr) in enumerate(((zs[0], big[1]),
                                                     (zs[1], big[0]))):
                        tp = tp_pool.tile([128, 128], F16, tag="tp", name="tp")
                        nc.tensor.transpose(tp[:], z_[:], ident16[:])
                        zt = ztp.tile([128, 128], F16, tag="zt")
                        nc.scalar.activation(zt[:], tp[:], AF.Copy)
                        t_ps = tps_pool.tile([128, H2], F32, tag="tps",
                                             name="tps")
                        for hf in range(2):
                            sl = slice(512 * hf, 512 * (hf + 1))
                            nc.tensor.matmul(t_ps[:, sl], zt[:], othr[:, sl],
                                             start=True, stop=True)
                        b_ = big[sd]
                        r_ = rs[sd]
                        til = b_[:, H2:2 * H2]
                        if sd == 0:
                            nc.scalar.activation(til, t_ps[:], AF.Copy,
                                                 scale=r_[:])
                        else:
                            nc.vector.tensor_scalar_mul(til, t_ps[:], r_[:])
                        outd = outA_d if sd == 0 else outB_d
                        nc.sync.dma_start(outd.ap()[n, :, :], b_[:])

                # software pipeline: stage_b runs two items behind stage_a so
                # its serial tail (zt->align->til->diff/prod->DMA) never sets
                # the loop period
                from collections import deque
                pend = deque()
                for n in range(PB):
                    pend.append((n, *stage_a(n)))
                    if len(pend) > 2:
                        stage_b(*pend.popleft())
                while pend:
                    stage_b(*pend.popleft())

    nc.compile()
    return nc


def _get_nc():
    if "nc" not in _CACHE:
        _CACHE["nc"] = _build()
    return _CACHE["nc"]


def prep_in_maps(inputs):
    A = np.asarray(inputs["A"])
    B = np.asarray(inputs["B"])
    embed = np.asarray(inputs["embed"], dtype=np.float32)
    # permute pytorch gate order [i,f,g,o] -> [f,i,g,o]
    perm = np.concatenate([np.arange(H, 2 * H), np.arange(0, H),
                           np.arange(2 * H, 3 * H), np.arange(3 * H, 4 * H)])
    wmat = {}
    for d in "fb":
        suf = "_" + d
        wih = np.asarray(inputs["Wih" + suf], dtype=np.float32)[perm]
        whh = np.asarray(inputs["Whh" + suf], dtype=np.float32)[perm]
        bias = (np.asarray(inputs["bih" + suf], dtype=np.float32)
                + np.asarray(inputs["bhh" + suf], dtype=np.float32))[perm]
        w = np.zeros((7 * 128, G4), dtype=np.float16)
        w[0:E] = wih.T.astype(np.float16)
        w[E] = bias.astype(np.float16)
        w[XC * 128:XC * 128 + H] = whh.T.astype(np.float16)
        wmat[d] = w

    xa = embed[A]    # [BSZ, T, E]
    xb = embed[B]

    in_maps = []
    for c in range(NCORES):
        sl = slice(PB * c, PB * (c + 1))
        xc = np.concatenate([xa[sl], xb[sl]], axis=0)          # [RW, T, E]
        xT = np.zeros((EA, RW * T), dtype=np.float16)
        xT[0:E] = xc.transpose(2, 0, 1).reshape(E, RW * T).astype(np.float16)
        xT[E] = 1.0
        in_maps.append({
            "xT": xT, "w_f": wmat["f"], "w_b": wmat["b"],
        })
    return in_maps


def kernel(**inputs):
    from concourse.bass_utils import run_bass_kernel_spmd

    in_maps = prep_in_maps(inputs)
    nc = _get_nc()
    res = run_bass_kernel_spmd(nc, in_maps, core_ids=list(range(NCORES)))

    def assemble(name):
        bt = np.concatenate(
            [res.results[c][name].astype(np.float32) for c in range(NCORES)],
            axis=0)                                    # [BSZ, T, 2*H2]
        bar, til = bt[:, :, 0:H2], bt[:, :, H2:2 * H2]
        return np.concatenate([bar, til, bar - til, bar * til], axis=2)

    return assemble("outA"), assemble("outB")


# revision 33
# speedup vs baseline: 3.3037x; 1.0148x over previous
"""Bass/Trainium2 kernel for nn_Encoder (embedding -> BiLSTM -> cross attention
-> enhancement).

Sharding: data-parallel over batch, 16 items per core on 8 NeuronCores (no
collectives). Per core the A and B sequences are stacked into 32 batch rows.

Layout: everything transposed vs the torch reference. Gates are computed as
g^T[2048, 32] = W_ih @ x_t^T + W_hh @ h^T with the gate dimension on SBUF
partitions and the 32 batch rows on the matmul moving dim, so every matmul
streams fp16 at 1 cycle/row and every vector/scalar op runs on all 128
partitions. The x-projection is inlined into the scan (3 extra fp16 K-chunks
per gate chunk; bias rides along as a ones-row of x), and is emitted XAHEAD
steps ahead of the recurrent matmuls to keep the PE array continuously busy
(full p-state). Each step+direction's PSUM bank is one accumulation group:
start=True on the bank's first (prefilled) matmul pend-zeroes the whole 2KB
region, stop=True on the last recurrent matmul. h_t is written by the DVE
directly into a persistent fp16 history tile [128, T, 4, 32] per direction,
which serves as (a) the next step's matmul rhs, (b) the feature-major
attention operands via strided APs, and (c) the source for the time-major
tiles (4 PE transposes batched per PSUM bank, one wide Act/DVE copy out).
The attention items run as a 2-deep software pipeline (E-matmuls+softmax two
items ahead of the z-transpose/align/output stage). The device ships
[bar | til] per item in one fp16 DMA; the host widens to f32 and derives the
bar-til / bar*til enhancement quarters.
"""

import numpy as np

V, E, H = 32000, 300, 512
BSZ, T = 128, 128
NCORES = 8
PB = BSZ // NCORES          # 16 batch items per core per sequence
RW = 2 * PB                 # 32 stacked rows (A items then B items)
G4 = 4 * H                  # 2048 gate width
H2 = 2 * H                  # 1024 bilstm output width
EA = 384                    # padded x feature dim (300 + bias row + zeros)
XC = 3                      # x K-chunks of 128
HC = 4                      # h K-chunks of 128
XAHEAD = 3                  # steps of x-projection prefill ahead of the scan

_CACHE = {}


def _build():
    import concourse.mybir as mybir
    import concourse.tile as tile
    from concourse import bacc
    from concourse.masks import make_identity

    F32 = mybir.dt.float32
    F16 = mybir.dt.float16
    AF = mybir.ActivationFunctionType
    ALU = mybir.AluOpType
    AX = mybir.AxisListType

    nc = bacc.Bacc("TRN2", target_bir_lowering=False, debug=False,
                   num_devices=NCORES)

    # Host-prepped inputs (see prep_in_maps): x^T with ones row for the bias,
    # and per-direction weights stacked as 7 K-chunks (3 wih+bias, 4 whh).
    xT_d = nc.dram_tensor("xT", [EA, RW * T], F16, kind="ExternalInput")
    w_d = {d: nc.dram_tensor(f"w_{d}", [7 * 128, G4], F16, kind="ExternalInput")
           for d in "fb"}
    # device ships [bar | til] per item; host derives bar-til and bar*til
    outA_d = nc.dram_tensor("outA", [PB, T, 2 * H2], F16, kind="ExternalOutput")
    outB_d = nc.dram_tensor("outB", [PB, T, 2 * H2], F16, kind="ExternalOutput")

    with tile.TileContext(nc) as tc:
        with tc.tile_pool(name="const", bufs=1) as const, \
             tc.tile_pool(name="hist", bufs=1) as histp:
            ident = const.tile([128, 128], F32)
            make_identity(nc, ident[:])
            ident16 = const.tile([128, 128], F16)
            nc.vector.tensor_copy(ident16[:], ident[:])

            # persistent fp16 h history, [128 d-in-chunk, T, h-chunk, batch]
            hist = {d: histp.tile([128, T, HC, RW], F16, name=f"hist_{d}")
                    for d in "fb"}

            # ---------------- Phase 1+2: fused projection + scan ----------
            with tc.tile_pool(name="wst", bufs=1) as wst, \
                 tc.tile_pool(name="xst", bufs=1) as xst, \
                 tc.tile_pool(name="cst", bufs=1) as cst, \
                 tc.tile_pool(name="gct", bufs=2) as gct, \
                 tc.tile_pool(name="ew", bufs=2) as ew, \
                 tc.tile_pool(name="gps", bufs=XAHEAD + 1, space="PSUM") as gps:
                # per-chunk loads ordered so the first x-matmuls can start
                # after just the first chunk of x and fwd weights
                wsb = {d: wst.tile([128, 7, G4], F16, name=f"w_{d}")
                       for d in "fb"}
                xk = xst.tile([128, XC, RW, T], F16, name="xk")
                for k in range(XC):
                    nc.sync.dma_start(
                        xk[:, k, :, :],
                        xT_d.ap()[k * 128:(k + 1) * 128, :])
                    for d in "fb":
                        nc.sync.dma_start(
                            wsb[d][:, k, :],
                            w_d[d].ap()[k * 128:(k + 1) * 128, :])
                for k in range(XC, 7):
                    for d in "fb":
                        nc.sync.dma_start(
                            wsb[d][:, k, :],
                            w_d[d].ap()[k * 128:(k + 1) * 128, :])
                c_st = {d: cst.tile([128, HC, RW], F16, name=f"c_{d}")
                        for d in "fb"}

                ps_tiles = {}

                # One accumulation group per PSUM bank per step: start=True on
                # the bank's first matmul marks the whole 2KB zero region, so
                # every gate chunk's first write lands zeroed; stop=True on
                # the bank's final matmul.  This keeps the x-projection
                # prefill (emitted XAHEAD steps early) in the same group as
                # the recurrent matmuls without interleaved-group violations.
                def emit_x(d, s):
                    tx = s if d == "f" else T - 1 - s
                    ps = gps.tile([128, 16, RW], F32, tag=f"ps{d}",
                                  name=f"ps{d}")
                    ps_tiles[(d, s)] = ps
                    last = s == 0
                    for gc in range(16):
                        for kx in range(XC):
                            nc.tensor.matmul(
                                ps[:, gc, :],
                                wsb[d][:, kx, gc * 128:(gc + 1) * 128],
                                xk[:, kx, :, tx],
                                start=(gc == 0 and kx == 0),
                                stop=(last and gc == 15 and kx == XC - 1))

                def emit_h(d, s):
                    tprev = (s - 1) if d == "f" else (T - s)
                    ps = ps_tiles[(d, s)]
                    hp = hist[d][:, tprev, :, :]
                    for gc in range(16):
                        for kc in range(HC):
                            nc.tensor.matmul(
                                ps[:, gc, :],
                                wsb[d][:, XC + kc, gc * 128:(gc + 1) * 128],
                                hp[:, kc, :],
                                start=False,
                                stop=(gc == 15 and kc == HC - 1))

                # Gate chunk layout (host-permuted): [f | i | g | o], 4 chunks
                # of 128 gate dims each.
                for s in range(min(XAHEAD, T)):
                    emit_x("f", s)
                    emit_x("b", s)
                for s in range(T):
                    da, db = ("f", "b") if s % 2 == 0 else ("b", "f")
                    if s >= 1:
                        emit_h(da, s)
                        emit_h(db, s)
                    if s + XAHEAD < T:
                        emit_x(da, s + XAHEAD)
                        emit_x(db, s + XAHEAD)
                    # Activation-queue order tuned so the f-chain's tanh(c)
                    # is not stuck behind the b-dir gate activations, and the
                    # b-chain exploits its natural slack (its matmuls sit in
                    # the second half of each PE burst).
                    ga, th_t = {}, {}
                    for d in "fb":
                        ps = ps_tiles.pop((d, s))
                        g = gct.tile([128, 16, RW], F16, tag=f"ga{d}",
                                     name=f"ga{d}")
                        ga[d] = (g, ps)
                        th_t[d] = ew.tile([128, HC, RW], F16, tag=f"th{d}",
                                          name=f"th{d}")

                    def act_sfi(d):
                        g, ps = ga[d]
                        nc.scalar.activation(g[:, 0:8, :], ps[:, 0:8, :],
                                             AF.Sigmoid)          # f, i

                    def act_tg(d):
                        g, ps = ga[d]
                        nc.scalar.activation(g[:, 8:12, :], ps[:, 8:12, :],
                                             AF.Tanh)             # g

                    def act_so(d):
                        g, ps = ga[d]
                        nc.scalar.activation(g[:, 12:16, :], ps[:, 12:16, :],
                                             AF.Sigmoid)          # o

                    def act_tc(d):
                        nc.scalar.activation(th_t[d][:], c_st[d][:], AF.Tanh)

                    def dve_q(d):
                        if s == 0:
                            return None
                        g, _ = ga[d]
                        q_ = ew.tile([128, HC, RW], F16, tag=f"q{d}",
                                     name=f"q{d}")
                        nc.vector.tensor_mul(q_[:], g[:, 0:4, :], c_st[d][:])
                        return q_

                    def dve_pc(d, q_):
                        g, _ = ga[d]
                        if s == 0:
                            nc.vector.tensor_mul(c_st[d][:], g[:, 4:8, :],
                                                 g[:, 8:12, :])   # c = i*g
                        else:
                            p_ = ew.tile([128, HC, RW], F16, tag=f"p{d}",
                                         name=f"p{d}")
                            nc.vector.tensor_mul(p_[:], g[:, 4:8, :],
                                                 g[:, 8:12, :])   # i*g
                            nc.vector.tensor_add(c_st[d][:], p_[:], q_[:])

                    def dve_h(d):
                        tx = s if d == "f" else T - 1 - s
                        nc.vector.tensor_mul(hist[d][:, tx, :, :],
                                             ga[d][0][:, 12:16, :], th_t[d][:])

                    # Alternate which direction leads each step so the act
                    # queue penalty of going second is shared evenly.
                    d1, d2 = ("f", "b") if s % 2 == 0 else ("b", "f")
                    act_sfi(d1)
                    act_tg(d1)
                    act_so(d1)
                    q1 = dve_q(d1)
                    dve_pc(d1, q1)
                    act_sfi(d2)
                    act_tc(d1)
                    q2 = dve_q(d2)
                    dve_h(d1)
                    act_tg(d2)
                    act_so(d2)
                    dve_pc(d2, q2)
                    act_tc(d2)
                    dve_h(d2)

            # ---------------- Phase 3: attention + enhancement -------------
            with tc.tile_pool(name="big", bufs=5) as bigp, \
                 tc.tile_pool(name="sm", bufs=6) as smp, \
                 tc.tile_pool(name="zt", bufs=2) as ztp, \
                 tc.tile_pool(name="eps", bufs=2, space="PSUM") as eps_pool, \
                 tc.tile_pool(name="tp", bufs=4, space="PSUM") as tp_pool, \
                 tc.tile_pool(name="tps", bufs=1, space="PSUM") as tps_pool:

                def fm(side, c8, n):
                    # feature-major [128 d, 128 t] strided view of history
                    d = "f" if c8 < 4 else "b"
                    return hist[d][:, :, c8 % 4, n if side == 0 else PB + n]

                def stage_a(n):
                    # E matmuls straight off the history (strided APs);
                    # e1/e2 pack into one PSUM bank
                    e12 = eps_pool.tile([128, 2, 128], F32, tag="e", name="e12")
                    e1, e2 = e12[:, 0, :], e12[:, 1, :]
                    for c8 in range(8):
                        nc.tensor.matmul(e1, fm(0, c8, n), fm(1, c8, n),
                                         start=(c8 == 0), stop=(c8 == 7))
                    for c8 in range(8):
                        nc.tensor.matmul(e2, fm(1, c8, n), fm(0, c8, n),
                                         start=(c8 == 0), stop=(c8 == 7))

                    # time-major tiles: 4 PE transposes batched per PSUM bank
                    # (one pending-zero group), then a single wide copy per
                    # bank alternating Act/DVE (GPSIMD cannot read PSUM).
                    # big layout: [bar | til]
                    big = [bigp.tile([128, 2 * H2], F16, tag=f"big{sd}",
                                     name=f"big{sd}") for sd in range(2)]
                    k = 0
                    for sd in range(2):
                        for half in range(2):
                            tp = tp_pool.tile([128, 4, 128], F16, tag="tp",
                                              name="tp")
                            for j in range(4):
                                nc.tensor.matmul(
                                    tp[:, j, :], fm(sd, half * 4 + j, n),
                                    ident16[:], is_transpose=True,
                                    start=(j == 0), stop=(j == 3))
                            dst = big[sd][:, half * 512:(half + 1) * 512]
                            if k % 2 == 0:
                                nc.scalar.activation(dst, tp[:, :, :], AF.Copy)
                            else:
                                nc.vector.tensor_copy(dst, tp[:, :, :])
                            k += 1

                    # softmaxes (row softmax of E and of E^T)
                    zs, rs = [], []
                    for eps in (e1, e2):
                        m_ = smp.tile([128, 1], F32, tag="m")
                        nc.vector.tensor_reduce(m_[:], eps[:], axis=AX.X,
                                                op=ALU.max, negate=True)
                        z_ = smp.tile([128, 128], F16, tag="z")
                        s_ = smp.tile([128, 1], F32, tag="s")
                        nc.scalar.activation(z_[:], eps[:], AF.Exp, bias=m_[:],
                                             accum_out=s_[:])
                        r_ = smp.tile([128, 1], F32, tag="r")
                        nc.vector.reciprocal(r_[:], s_[:])
                        zs.append(z_)
                        rs.append(r_)
                    return zs, rs, big

                def stage_b(n, zs, rs, big):
                    # z transposes + align matmuls + enhancement + out DMA
                    for sd, (z_, othr) in enumerate(((zs[0], big[1]),
                                                     (zs[1], big[0]))):
                        tp = tp_pool.tile([128, 128], F16, tag="tp", name="tp")
                        nc.tensor.transpose(tp[:], z_[:], ident16[:])
                        zt = ztp.tile([128, 128], F16, tag="zt")
                        nc.scalar.activation(zt[:], tp[:], AF.Copy)
                        t_ps = tps_pool.tile([128, H2], F32, tag="tps",
                                             name="tps")
                        for hf in range(2):
                            sl = slice(512 * hf, 512 * (hf + 1))
                            nc.tensor.matmul(t_ps[:, sl], zt[:], othr[:, sl],
                                             start=True, stop=True)
                        b_ = big[sd]
                        r_ = rs[sd]
                        til = b_[:, H2:2 * H2]
                        if sd == 0:
                            nc.scalar.activation(til, t_ps[:], AF.Copy,
                                                 scale=r_[:])
                        else:
                            nc.vector.tensor_scalar_mul(til, t_ps[:], r_[:])
                        outd = outA_d if sd == 0 else outB_d
                        nc.sync.dma_start(outd.ap()[n, :, :], b_[:])

                # software pipeline: stage_b runs two items behind stage_a so
                # its serial tail (zt->align->til->diff/prod->DMA) never sets
                # the loop period
                from collections import deque
                pend = deque()
                for n in range(PB):
                    pend.append((n, *stage_a(n)))
                    if len(pend) > 2:
                        stage_b(*pend.popleft())
                while pend:
                    stage_b(*pend.popleft())

    nc.compile()
    return nc


def _get_nc():
    if "nc" not in _CACHE:
        _CACHE["nc"] = _build()
    return _CACHE["nc"]


def prep_in_maps(inputs):
    A = np.asarray(inputs["A"])
    B = np.asarray(inputs["B"])
    embed = np.asarray(inputs["embed"], dtype=np.float32)
    # permute pytorch gate order [i,f,g,o] -> [f,i,g,o]
    perm = np.concatenate([np.arange(H, 2 * H), np.arange(0, H),
                           np.arange(2 * H, 3 * H), np.arange(3 * H, 4 * H)])
    wmat = {}
    for d in "fb":
        suf = "_" + d
        wih = np.asarray(inputs["Wih" + suf], dtype=np.float32)[perm]
        whh = np.asarray(inputs["Whh" + suf], dtype=np.float32)[perm]
        bias = (np.asarray(inputs["bih" + suf], dtype=np.float32)
                + np.asarray(inputs["bhh" + suf], dtype=np.float32))[perm]
        w = np.zeros((7 * 128, G4), dtype=np.float16)
        w[0:E] = wih.T.astype(np.float16)
        w[E] = bias.astype(np.float16)
        w[XC * 128:XC * 128 + H] = whh.T.astype(np.float16)
        wmat[d] = w

    xa = embed[A]    # [BSZ, T, E]
    xb = embed[B]

    in_maps = []
    for c in range(NCORES):
        sl = slice(PB * c, PB * (c + 1))
        xc = np.concatenate([xa[sl], xb[sl]], axis=0)          # [RW, T, E]
        xT = np.zeros((EA, RW * T), dtype=np.float16)
        xT[0:E] = xc.transpose(2, 0, 1).reshape(E, RW * T).astype(np.float16)
        xT[E] = 1.0
        in_maps.append({
            "xT": xT, "w_f": wmat["f"], "w_b": wmat["b"],
        })
    return in_maps


def kernel(**inputs):
    from concourse.bass_utils import run_bass_kernel_spmd

    in_maps = prep_in_maps(inputs)
    nc = _get_nc()
    res = run_bass_kernel_spmd(nc, in_maps, core_ids=list(range(NCORES)))

    def assemble(name):
        bt = np.concatenate(
            [res.results[c][name].astype(np.float32) for c in range(NCORES)],
            axis=0)                                    # [BSZ, T, 2*H2]
        bar, til = bt[:, :, 0:H2], bt[:, :, H2:2 * H2]
        return np.concatenate([bar, til, bar - til, bar * til], axis=2)

    return assemble("outA"), assemble("outB")


# revision 41
# speedup vs baseline: 3.3232x; 1.0059x over previous
"""Bass/Trainium2 kernel for nn_Encoder (embedding -> BiLSTM -> cross attention
-> enhancement).

Sharding: data-parallel over batch, 16 items per core on 8 NeuronCores (no
collectives). Per core the A and B sequences are stacked into 32 batch rows.

Layout: everything transposed vs the torch reference. Gates are computed as
g^T[2048, 32] = W_ih @ x_t^T + W_hh @ h^T with the gate dimension on SBUF
partitions and the 32 batch rows on the matmul moving dim, so every matmul
streams fp16 at 1 cycle/row and every vector/scalar op runs on all 128
partitions. The x-projection is inlined into the scan (3 extra fp16 K-chunks
per gate chunk; bias rides along as a ones-row of x), and is emitted XAHEAD
steps ahead of the recurrent matmuls to keep the PE array continuously busy
(full p-state). Each step+direction's PSUM bank is one accumulation group:
start=True on the bank's first (prefilled) matmul pend-zeroes the whole 2KB
region, stop=True on the last recurrent matmul. h_t is written by the DVE
directly into a persistent fp16 history tile [128, T, 4, 32] per direction,
which serves as (a) the next step's matmul rhs, (b) the feature-major
attention operands via strided APs, and (c) the source for the time-major
tiles (4 PE transposes batched per PSUM bank, one wide Act/DVE copy out).
The attention items run as a 2-deep software pipeline (E-matmuls+softmax two
items ahead of the z-transpose/align/output stage). The device ships
[bar | til] per item in one fp16 DMA; the host widens to f32 and derives the
bar-til / bar*til enhancement quarters.
"""

import numpy as np

V, E, H = 32000, 300, 512
BSZ, T = 128, 128
NCORES = 8
PB = BSZ // NCORES          # 16 batch items per core per sequence
RW = 2 * PB                 # 32 stacked rows (A items then B items)
G4 = 4 * H                  # 2048 gate width
H2 = 2 * H                  # 1024 bilstm output width
EA = 384                    # padded x feature dim (300 + bias row + zeros)
XC = 3                      # x K-chunks of 128
HC = 4                      # h K-chunks of 128
XAHEAD = 3                  # steps of x-projection prefill ahead of the scan

_CACHE = {}


def _build():
    import concourse.mybir as mybir
    import concourse.tile as tile
    from concourse import bacc
    from concourse.masks import make_identity

    F32 = mybir.dt.float32
    F16 = mybir.dt.float16
    AF = mybir.ActivationFunctionType
    ALU = mybir.AluOpType
    AX = mybir.AxisListType

    nc = bacc.Bacc("TRN2", target_bir_lowering=False, debug=False,
                   num_devices=NCORES)

    # Host-prepped inputs (see prep_in_maps): x^T with ones row for the bias,
    # and per-direction weights stacked as K-chunks (wih+bias rows 0:301,
    # then whh).  Zero padding rows are not shipped: the third x K-chunk is
    # only 45 rows (300 features + ones row - 256).
    KL = E + 1 - 2 * 128  # 45
    xT_d = nc.dram_tensor("xT", [2 * 128 + KL, RW * T], F16,
                          kind="ExternalInput")
    w_d = {d: nc.dram_tensor(f"w_{d}", [2 * 128 + KL + 4 * 128, G4], F16,
                             kind="ExternalInput")
           for d in "fb"}
    # device ships [bar | til] per item; host derives bar-til and bar*til
    outA_d = nc.dram_tensor("outA", [PB, T, 2 * H2], F16, kind="ExternalOutput")
    outB_d = nc.dram_tensor("outB", [PB, T, 2 * H2], F16, kind="ExternalOutput")

    with tile.TileContext(nc) as tc:
        with tc.tile_pool(name="const", bufs=1) as const, \
             tc.tile_pool(name="hist", bufs=1) as histp:
            ident = const.tile([128, 128], F32)
            make_identity(nc, ident[:])
            ident16 = const.tile([128, 128], F16)
            nc.vector.tensor_copy(ident16[:], ident[:])

            # persistent fp16 h history, [128 d-in-chunk, T, h-chunk, batch]
            hist = {d: histp.tile([128, T, HC, RW], F16, name=f"hist_{d}")
                    for d in "fb"}

            # ---------------- Phase 1+2: fused projection + scan ----------
            with tc.tile_pool(name="wst", bufs=1) as wst, \
                 tc.tile_pool(name="xst", bufs=1) as xst, \
                 tc.tile_pool(name="cst", bufs=1) as cst, \
                 tc.tile_pool(name="gct", bufs=2) as gct, \
                 tc.tile_pool(name="ew", bufs=2) as ew, \
                 tc.tile_pool(name="gps", bufs=XAHEAD + 1, space="PSUM") as gps:
                # per-chunk loads ordered so the first x-matmuls can start
                # after just the first chunk of x and fwd weights
                wsb = {d: wst.tile([128, 7, G4], F16, name=f"w_{d}")
                       for d in "fb"}
                xk = xst.tile([128, XC, RW, T], F16, name="xk")
                xlen = [128, 128, KL]
                for k in range(XC):
                    o = k * 128
                    nc.sync.dma_start(
                        xk[0:xlen[k], k, :, :],
                        xT_d.ap()[o:o + xlen[k], :])
                    for d in "fb":
                        nc.sync.dma_start(
                            wsb[d][0:xlen[k], k, :],
                            w_d[d].ap()[o:o + xlen[k], :])
                for k in range(XC, 7):
                    o = 2 * 128 + KL + (k - XC) * 128
                    for d in "fb":
                        nc.sync.dma_start(
                            wsb[d][:, k, :],
                            w_d[d].ap()[o:o + 128, :])
                c_st = {d: cst.tile([128, HC, RW], F16, name=f"c_{d}")
                        for d in "fb"}

                ps_tiles = {}

                # One accumulation group per PSUM bank per step: start=True on
                # the bank's first matmul marks the whole 2KB zero region, so
                # every gate chunk's first write lands zeroed; stop=True on
                # the bank's final matmul.  This keeps the x-projection
                # prefill (emitted XAHEAD steps early) in the same group as
                # the recurrent matmuls without interleaved-group violations.
                def emit_x(d, s):
                    tx = s if d == "f" else T - 1 - s
                    ps = gps.tile([128, 16, RW], F32, tag=f"ps{d}",
                                  name=f"ps{d}")
                    ps_tiles[(d, s)] = ps
                    last = s == 0
                    for gc in range(16):
                        for kx in range(XC):
                            kk = 128 if kx < 2 else KL
                            nc.tensor.matmul(
                                ps[:, gc, :],
                                wsb[d][0:kk, kx, gc * 128:(gc + 1) * 128],
                                xk[0:kk, kx, :, tx],
                                start=(gc == 0 and kx == 0),
                                stop=(last and gc == 15 and kx == XC - 1))

                def emit_h(d, s):
                    tprev = (s - 1) if d == "f" else (T - s)
                    ps = ps_tiles[(d, s)]
                    hp = hist[d][:, tprev, :, :]
                    for gc in range(16):
                        for kc in range(HC):
                            nc.tensor.matmul(
                                ps[:, gc, :],
                                wsb[d][:, XC + kc, gc * 128:(gc + 1) * 128],
                                hp[:, kc, :],
                                start=False,
                                stop=(gc == 15 and kc == HC - 1))

                # Gate chunk layout (host-permuted): [f | i | g | o], 4 chunks
                # of 128 gate dims each.
                for s in range(min(XAHEAD, T)):
                    emit_x("f", s)
                    emit_x("b", s)
                for s in range(T):
                    da, db = ("f", "b") if s % 2 == 0 else ("b", "f")
                    if s >= 1:
                        emit_h(da, s)
                        emit_h(db, s)
                    if s + XAHEAD < T:
                        emit_x(da, s + XAHEAD)
                        emit_x(db, s + XAHEAD)
                    # Activation-queue order tuned so the f-chain's tanh(c)
                    # is not stuck behind the b-dir gate activations, and the
                    # b-chain exploits its natural slack (its matmuls sit in
                    # the second half of each PE burst).
                    ga, th_t = {}, {}
                    for d in "fb":
                        ps = ps_tiles.pop((d, s))
                        g = gct.tile([128, 16, RW], F16, tag=f"ga{d}",
                                     name=f"ga{d}")
                        ga[d] = (g, ps)
                        th_t[d] = ew.tile([128, HC, RW], F16, tag=f"th{d}",
                                          name=f"th{d}")

                    def act_sfi(d):
                        g, ps = ga[d]
                        nc.scalar.activation(g[:, 0:8, :], ps[:, 0:8, :],
                                             AF.Sigmoid)          # f, i

                    def act_tg(d):
                        g, ps = ga[d]
                        nc.scalar.activation(g[:, 8:12, :], ps[:, 8:12, :],
                                             AF.Tanh)             # g

                    def act_so(d):
                        g, ps = ga[d]
                        nc.scalar.activation(g[:, 12:16, :], ps[:, 12:16, :],
                                             AF.Sigmoid)          # o

                    def act_tc(d):
                        nc.scalar.activation(th_t[d][:], c_st[d][:], AF.Tanh)

                    def dve_q(d):
                        if s == 0:
                            return None
                        g, _ = ga[d]
                        q_ = ew.tile([128, HC, RW], F16, tag=f"q{d}",
                                     name=f"q{d}")
                        nc.vector.tensor_mul(q_[:], g[:, 0:4, :], c_st[d][:])
                        return q_

                    def dve_pc(d, q_):
                        g, _ = ga[d]
                        if s == 0:
                            nc.vector.tensor_mul(c_st[d][:], g[:, 4:8, :],
                                                 g[:, 8:12, :])   # c = i*g
                        else:
                            p_ = ew.tile([128, HC, RW], F16, tag=f"p{d}",
                                         name=f"p{d}")
                            nc.vector.tensor_mul(p_[:], g[:, 4:8, :],
                                                 g[:, 8:12, :])   # i*g
                            nc.vector.tensor_add(c_st[d][:], p_[:], q_[:])

                    def dve_h(d):
                        tx = s if d == "f" else T - 1 - s
                        nc.vector.tensor_mul(hist[d][:, tx, :, :],
                                             ga[d][0][:, 12:16, :], th_t[d][:])

                    # Alternate which direction leads each step so the act
                    # queue penalty of going second is shared evenly.
                    d1, d2 = ("f", "b") if s % 2 == 0 else ("b", "f")
                    act_sfi(d1)
                    act_tg(d1)
                    act_so(d1)
                    q1 = dve_q(d1)
                    dve_pc(d1, q1)
                    act_sfi(d2)
                    act_tc(d1)
                    q2 = dve_q(d2)
                    dve_h(d1)
                    act_tg(d2)
                    act_so(d2)
                    dve_pc(d2, q2)
                    act_tc(d2)
                    dve_h(d2)

            # ---------------- Phase 3: attention + enhancement -------------
            with tc.tile_pool(name="big", bufs=5) as bigp, \
                 tc.tile_pool(name="sm", bufs=6) as smp, \
                 tc.tile_pool(name="zt", bufs=2) as ztp, \
                 tc.tile_pool(name="eps", bufs=2, space="PSUM") as eps_pool, \
                 tc.tile_pool(name="tp", bufs=4, space="PSUM") as tp_pool, \
                 tc.tile_pool(name="tps", bufs=1, space="PSUM") as tps_pool:

                def fm(side, c8, n):
                    # feature-major [128 d, 128 t] strided view of history
                    d = "f" if c8 < 4 else "b"
                    return hist[d][:, :, c8 % 4, n if side == 0 else PB + n]

                def stage_a(n):
                    # E matmuls straight off the history (strided APs);
                    # e1/e2 pack into one PSUM bank
                    e12 = eps_pool.tile([128, 2, 128], F32, tag="e", name="e12")
                    e1, e2 = e12[:, 0, :], e12[:, 1, :]
                    for c8 in range(8):
                        nc.tensor.matmul(e1, fm(0, c8, n), fm(1, c8, n),
                                         start=(c8 == 0), stop=(c8 == 7))
                    for c8 in range(8):
                        nc.tensor.matmul(e2, fm(1, c8, n), fm(0, c8, n),
                                         start=(c8 == 0), stop=(c8 == 7))

                    # time-major tiles: 4 PE transposes batched per PSUM bank
                    # (one pending-zero group), then a single wide copy per
                    # bank alternating Act/DVE (GPSIMD cannot read PSUM).
                    # big layout: [bar | til]
                    big = [bigp.tile([128, 2 * H2], F16, tag=f"big{sd}",
                                     name=f"big{sd}") for sd in range(2)]
                    k = 0
                    for sd in range(2):
                        for half in range(2):
                            tp = tp_pool.tile([128, 4, 128], F16, tag="tp",
                                              name="tp")
                            for j in range(4):
                                nc.tensor.matmul(
                                    tp[:, j, :], fm(sd, half * 4 + j, n),
                                    ident16[:], is_transpose=True,
                                    start=(j == 0), stop=(j == 3))
                            dst = big[sd][:, half * 512:(half + 1) * 512]
                            if k % 2 == 0:
                                nc.scalar.activation(dst, tp[:, :, :], AF.Copy)
                            else:
                                nc.vector.tensor_copy(dst, tp[:, :, :])
                            k += 1

                    # softmaxes (row softmax of E and of E^T)
                    zs, rs = [], []
                    for eps in (e1, e2):
                        m_ = smp.tile([128, 1], F32, tag="m")
                        nc.vector.tensor_reduce(m_[:], eps[:], axis=AX.X,
                                                op=ALU.max, negate=True)
                        z_ = smp.tile([128, 128], F16, tag="z")
                        nc.scalar.activation(z_[:], eps[:], AF.Exp, bias=m_[:])
                        s_ = smp.tile([128, 1], F32, tag="s")
                        nc.vector.tensor_reduce(s_[:], z_[:], axis=AX.X,
                                                op=ALU.add)
                        r_ = smp.tile([128, 1], F32, tag="r")
                        nc.vector.reciprocal(r_[:], s_[:])
                        zs.append(z_)
                        rs.append(r_)
                    return zs, rs, big

                def stage_b(n, zs, rs, big):
                    # z transposes + align matmuls + enhancement + out DMA
                    for sd, (z_, othr) in enumerate(((zs[0], big[1]),
                                                     (zs[1], big[0]))):
                        tp = tp_pool.tile([128, 128], F16, tag="tp", name="tp")
                        nc.tensor.transpose(tp[:], z_[:], ident16[:])
                        zt = ztp.tile([128, 128], F16, tag="zt")
                        nc.scalar.activation(zt[:], tp[:], AF.Copy)
                        t_ps = tps_pool.tile([128, H2], F32, tag="tps",
                                             name="tps")
                        for hf in range(2):
                            sl = slice(512 * hf, 512 * (hf + 1))
                            nc.tensor.matmul(t_ps[:, sl], zt[:], othr[:, sl],
                                             start=True, stop=True)
                        b_ = big[sd]
                        r_ = rs[sd]
                        til = b_[:, H2:2 * H2]
                        if sd == 0:
                            nc.scalar.activation(til, t_ps[:], AF.Copy,
                                                 scale=r_[:])
                        else:
                            nc.vector.tensor_scalar_mul(til, t_ps[:], r_[:])
                        outd = outA_d if sd == 0 else outB_d
                        nc.sync.dma_start(outd.ap()[n, :, :], b_[:])

                # software pipeline: stage_b runs two items behind stage_a so
                # its serial tail (zt->align->til->diff/prod->DMA) never sets
                # the loop period
                from collections import deque
                pend = deque()
                for n in range(PB):
                    pend.append((n, *stage_a(n)))
                    if len(pend) > 2:
                        stage_b(*pend.popleft())
                while pend:
                    stage_b(*pend.popleft())

    nc.compile()
    return nc


def _get_nc():
    if "nc" not in _CACHE:
        _CACHE["nc"] = _build()
    return _CACHE["nc"]


def prep_in_maps(inputs):
    A = np.asarray(inputs["A"])
    B = np.asarray(inputs["B"])
    embed = np.asarray(inputs["embed"], dtype=np.float32)
    # permute pytorch gate order [i,f,g,o] -> [f,i,g,o]
    perm = np.concatenate([np.arange(H, 2 * H), np.arange(0, H),
                           np.arange(2 * H, 3 * H), np.arange(3 * H, 4 * H)])
    wmat = {}
    for d in "fb":
        suf = "_" + d
        wih = np.asarray(inputs["Wih" + suf], dtype=np.float32)[perm]
        whh = np.asarray(inputs["Whh" + suf], dtype=np.float32)[perm]
        bias = (np.asarray(inputs["bih" + suf], dtype=np.float32)
                + np.asarray(inputs["bhh" + suf], dtype=np.float32))[perm]
        w = np.empty((E + 1 + H, G4), dtype=np.float16)
        w[0:E] = wih.T.astype(np.float16)
        w[E] = bias.astype(np.float16)
        w[E + 1:] = whh.T.astype(np.float16)
        wmat[d] = w

    xa = embed[A]    # [BSZ, T, E]
    xb = embed[B]

    in_maps = []
    for c in range(NCORES):
        sl = slice(PB * c, PB * (c + 1))
        xc = np.concatenate([xa[sl], xb[sl]], axis=0)          # [RW, T, E]
        xT = np.empty((E + 1, RW * T), dtype=np.float16)
        xT[0:E] = xc.transpose(2, 0, 1).reshape(E, RW * T).astype(np.float16)
        xT[E] = 1.0
        in_maps.append({
            "xT": xT, "w_f": wmat["f"], "w_b": wmat["b"],
        })
    return in_maps


def kernel(**inputs):
    from concourse.bass_utils import run_bass_kernel_spmd

    in_maps = prep_in_maps(inputs)
    nc = _get_nc()
    res = run_bass_kernel_spmd(nc, in_maps, core_ids=list(range(NCORES)))

    def assemble(name):
        bt = np.concatenate(
            [res.results[c][name].astype(np.float32) for c in range(NCORES)],
            axis=0)                                    # [BSZ, T, 2*H2]
        bar, til = bt[:, :, 0:H2], bt[:, :, H2:2 * H2]
        return np.concatenate([bar, til, bar - til, bar * til], axis=2)

    return assemble("outA"), assemble("outB")


# revision 42
# speedup vs baseline: 3.3708x; 1.0143x over previous
"""Bass/Trainium2 kernel for nn_Encoder (embedding -> BiLSTM -> cross attention
-> enhancement).

Sharding: data-parallel over batch, 16 items per core on 8 NeuronCores (no
collectives). Per core the A and B sequences are stacked into 32 batch rows.

Layout: everything transposed vs the torch reference. Gates are computed as
g^T[2048, 32] = W_ih @ x_t^T + W_hh @ h^T with the gate dimension on SBUF
partitions and the 32 batch rows on the matmul moving dim, so every matmul
streams fp16 at 1 cycle/row and every vector/scalar op runs on all 128
partitions. The x-projection is inlined into the scan (3 extra fp16 K-chunks
per gate chunk; bias rides along as a ones-row of x), and is emitted XAHEAD
steps ahead of the recurrent matmuls to keep the PE array continuously busy
(full p-state). Each step+direction's PSUM bank is one accumulation group:
start=True on the bank's first (prefilled) matmul pend-zeroes the whole 2KB
region, stop=True on the last recurrent matmul. h_t is written by the DVE
directly into a persistent fp16 history tile [128, T, 4, 32] per direction,
which serves as (a) the next step's matmul rhs, (b) the feature-major
attention operands via strided APs, and (c) the source for the time-major
tiles (4 PE transposes batched per PSUM bank, one wide Act/DVE copy out).
The attention items run as a 2-deep software pipeline (E-matmuls+softmax two
items ahead of the z-transpose/align/output stage). The device ships
[bar | til] per item in one fp16 DMA; the host widens to f32 and derives the
bar-til / bar*til enhancement quarters.
"""

import numpy as np

V, E, H = 32000, 300, 512
BSZ, T = 128, 128
NCORES = 8
PB = BSZ // NCORES          # 16 batch items per core per sequence
RW = 2 * PB                 # 32 stacked rows (A items then B items)
G4 = 4 * H                  # 2048 gate width
H2 = 2 * H                  # 1024 bilstm output width
EA = 384                    # padded x feature dim (300 + bias row + zeros)
XC = 3                      # x K-chunks of 128
HC = 4                      # h K-chunks of 128
XAHEAD = 3                  # steps of x-projection prefill ahead of the scan

_CACHE = {}


def _build():
    import concourse.mybir as mybir
    import concourse.tile as tile
    from concourse import bacc
    from concourse.masks import make_identity

    F32 = mybir.dt.float32
    F16 = mybir.dt.float16
    AF = mybir.ActivationFunctionType
    ALU = mybir.AluOpType
    AX = mybir.AxisListType

    nc = bacc.Bacc("TRN2", target_bir_lowering=False, debug=False,
                   num_devices=NCORES)

    # Host-prepped inputs (see prep_in_maps): x^T with ones row for the bias,
    # and per-direction weights stacked as K-chunks (wih+bias rows 0:301,
    # then whh).  Zero padding rows are not shipped: the third x K-chunk is
    # only 45 rows (300 features + ones row - 256).
    KL = E + 1 - 2 * 128  # 45
    xT_d = nc.dram_tensor("xT", [2 * 128 + KL, RW * T], F16,
                          kind="ExternalInput")
    w_d = {d: nc.dram_tensor(f"w_{d}", [2 * 128 + KL + 4 * 128, G4], F16,
                             kind="ExternalInput")
           for d in "fb"}
    # device ships [bar | til] per item; host derives bar-til and bar*til
    outA_d = nc.dram_tensor("outA", [PB, T, 2 * H2], F16, kind="ExternalOutput")
    outB_d = nc.dram_tensor("outB", [PB, T, 2 * H2], F16, kind="ExternalOutput")

    with tile.TileContext(nc) as tc:
        with tc.tile_pool(name="const", bufs=1) as const, \
             tc.tile_pool(name="hist", bufs=1) as histp:
            ident = const.tile([128, 128], F32)
            make_identity(nc, ident[:])
            ident16 = const.tile([128, 128], F16)
            nc.vector.tensor_copy(ident16[:], ident[:])

            # persistent fp16 h history, [128 d-in-chunk, T, h-chunk, batch]
            hist = {d: histp.tile([128, T, HC, RW], F16, name=f"hist_{d}")
                    for d in "fb"}

            # ---------------- Phase 1+2: fused projection + scan ----------
            with tc.tile_pool(name="wst", bufs=1) as wst, \
                 tc.tile_pool(name="xst", bufs=1) as xst, \
                 tc.tile_pool(name="cst", bufs=1) as cst, \
                 tc.tile_pool(name="gct", bufs=2) as gct, \
                 tc.tile_pool(name="ew", bufs=2) as ew, \
                 tc.tile_pool(name="gps", bufs=XAHEAD + 1, space="PSUM") as gps:
                # per-chunk loads ordered so the first x-matmuls can start
                # after just the first chunk of x and fwd weights
                wsb = {d: wst.tile([128, 7, G4], F16, name=f"w_{d}")
                       for d in "fb"}
                xk = xst.tile([128, XC, RW, T], F16, name="xk")
                xlen = [128, 128, KL]
                for k in range(XC):
                    o = k * 128
                    nc.sync.dma_start(
                        xk[0:xlen[k], k, :, :],
                        xT_d.ap()[o:o + xlen[k], :])
                    for d in "fb":
                        nc.sync.dma_start(
                            wsb[d][0:xlen[k], k, :],
                            w_d[d].ap()[o:o + xlen[k], :])
                for k in range(XC, 7):
                    o = 2 * 128 + KL + (k - XC) * 128
                    for d in "fb":
                        nc.sync.dma_start(
                            wsb[d][:, k, :],
                            w_d[d].ap()[o:o + 128, :])
                c_st = {d: cst.tile([128, HC, RW], F16, name=f"c_{d}")
                        for d in "fb"}

                ps_tiles = {}

                # One accumulation group per PSUM bank per step: start=True on
                # the bank's first matmul marks the whole 2KB zero region, so
                # every gate chunk's first write lands zeroed; stop=True on
                # the bank's final matmul.  This keeps the x-projection
                # prefill (emitted XAHEAD steps early) in the same group as
                # the recurrent matmuls without interleaved-group violations.
                def emit_x(d, s):
                    tx = s if d == "f" else T - 1 - s
                    ps = gps.tile([128, 16, RW], F32, tag=f"ps{d}",
                                  name=f"ps{d}")
                    ps_tiles[(d, s)] = ps
                    last = s == 0
                    for gc in range(16):
                        for kx in range(XC):
                            kk = 128 if kx < 2 else KL
                            nc.tensor.matmul(
                                ps[:, gc, :],
                                wsb[d][0:kk, kx, gc * 128:(gc + 1) * 128],
                                xk[0:kk, kx, :, tx],
                                start=(gc == 0 and kx == 0),
                                stop=(last and gc == 15 and kx == XC - 1))

                def emit_h(d, s):
                    tprev = (s - 1) if d == "f" else (T - s)
                    ps = ps_tiles[(d, s)]
                    hp = hist[d][:, tprev, :, :]
                    for gc in range(16):
                        for kc in range(HC):
                            nc.tensor.matmul(
                                ps[:, gc, :],
                                wsb[d][:, XC + kc, gc * 128:(gc + 1) * 128],
                                hp[:, kc, :],
                                start=False,
                                stop=(gc == 15 and kc == HC - 1))

                # Gate chunk layout (host-permuted): [f | i | o | g], 4 chunks
                # of 128 gate dims each; sigmoids contiguous in chunks 0-11.
                for s in range(min(XAHEAD, T)):
                    emit_x("f", s)
                    emit_x("b", s)
                for s in range(T):
                    da, db = ("f", "b") if s % 2 == 0 else ("b", "f")
                    if s >= 1:
                        emit_h(da, s)
                        emit_h(db, s)
                    if s + XAHEAD < T:
                        emit_x(da, s + XAHEAD)
                        emit_x(db, s + XAHEAD)
                    # Activation-queue order tuned so the f-chain's tanh(c)
                    # is not stuck behind the b-dir gate activations, and the
                    # b-chain exploits its natural slack (its matmuls sit in
                    # the second half of each PE burst).
                    ga, th_t = {}, {}
                    for d in "fb":
                        ps = ps_tiles.pop((d, s))
                        g = gct.tile([128, 16, RW], F16, tag=f"ga{d}",
                                     name=f"ga{d}")
                        ga[d] = (g, ps)
                        th_t[d] = ew.tile([128, HC, RW], F16, tag=f"th{d}",
                                          name=f"th{d}")

                    def act_sfi(d):
                        g, ps = ga[d]
                        nc.scalar.activation(g[:, 0:12, :], ps[:, 0:12, :],
                                             AF.Sigmoid)          # f, i, o

                    def act_tg(d):
                        g, ps = ga[d]
                        nc.scalar.activation(g[:, 12:16, :], ps[:, 12:16, :],
                                             AF.Tanh)             # g

                    def act_so(d):
                        pass

                    def act_tc(d):
                        nc.scalar.activation(th_t[d][:], c_st[d][:], AF.Tanh)

                    def dve_q(d):
                        if s == 0:
                            return None
                        g, _ = ga[d]
                        q_ = ew.tile([128, HC, RW], F16, tag=f"q{d}",
                                     name=f"q{d}")
                        nc.vector.tensor_mul(q_[:], g[:, 0:4, :], c_st[d][:])
                        return q_

                    def dve_pc(d, q_):
                        g, _ = ga[d]
                        if s == 0:
                            nc.vector.tensor_mul(c_st[d][:], g[:, 4:8, :],
                                                 g[:, 12:16, :])  # c = i*g
                        else:
                            p_ = ew.tile([128, HC, RW], F16, tag=f"p{d}",
                                         name=f"p{d}")
                            nc.vector.tensor_mul(p_[:], g[:, 4:8, :],
                                                 g[:, 12:16, :])  # i*g
                            nc.vector.tensor_add(c_st[d][:], p_[:], q_[:])

                    def dve_h(d):
                        tx = s if d == "f" else T - 1 - s
                        nc.vector.tensor_mul(hist[d][:, tx, :, :],
                                             ga[d][0][:, 8:12, :], th_t[d][:])

                    # Alternate which direction leads each step so the act
                    # queue penalty of going second is shared evenly.
                    d1, d2 = ("f", "b") if s % 2 == 0 else ("b", "f")
                    act_sfi(d1)
                    act_tg(d1)
                    act_so(d1)
                    q1 = dve_q(d1)
                    dve_pc(d1, q1)
                    act_sfi(d2)
                    act_tc(d1)
                    q2 = dve_q(d2)
                    dve_h(d1)
                    act_tg(d2)
                    act_so(d2)
                    dve_pc(d2, q2)
                    act_tc(d2)
                    dve_h(d2)

            # ---------------- Phase 3: attention + enhancement -------------
            with tc.tile_pool(name="big", bufs=5) as bigp, \
                 tc.tile_pool(name="sm", bufs=6) as smp, \
                 tc.tile_pool(name="zt", bufs=2) as ztp, \
                 tc.tile_pool(name="eps", bufs=2, space="PSUM") as eps_pool, \
                 tc.tile_pool(name="tp", bufs=4, space="PSUM") as tp_pool, \
                 tc.tile_pool(name="tps", bufs=1, space="PSUM") as tps_pool:

                def fm(side, c8, n):
                    # feature-major [128 d, 128 t] strided view of history
                    d = "f" if c8 < 4 else "b"
                    return hist[d][:, :, c8 % 4, n if side == 0 else PB + n]

                def stage_a(n):
                    # E matmuls straight off the history (strided APs);
                    # e1/e2 pack into one PSUM bank
                    e12 = eps_pool.tile([128, 2, 128], F32, tag="e", name="e12")
                    e1, e2 = e12[:, 0, :], e12[:, 1, :]
                    for c8 in range(8):
                        nc.tensor.matmul(e1, fm(0, c8, n), fm(1, c8, n),
                                         start=(c8 == 0), stop=(c8 == 7))
                    for c8 in range(8):
                        nc.tensor.matmul(e2, fm(1, c8, n), fm(0, c8, n),
                                         start=(c8 == 0), stop=(c8 == 7))

                    # time-major tiles: 4 PE transposes batched per PSUM bank
                    # (one pending-zero group), then a single wide copy per
                    # bank alternating Act/DVE (GPSIMD cannot read PSUM).
                    # big layout: [bar | til]
                    big = [bigp.tile([128, 2 * H2], F16, tag=f"big{sd}",
                                     name=f"big{sd}") for sd in range(2)]
                    k = 0
                    for sd in range(2):
                        for half in range(2):
                            tp = tp_pool.tile([128, 4, 128], F16, tag="tp",
                                              name="tp")
                            for j in range(4):
                                nc.tensor.matmul(
                                    tp[:, j, :], fm(sd, half * 4 + j, n),
                                    ident16[:], is_transpose=True,
                                    start=(j == 0), stop=(j == 3))
                            dst = big[sd][:, half * 512:(half + 1) * 512]
                            if k % 2 == 0:
                                nc.scalar.activation(dst, tp[:, :, :], AF.Copy)
                            else:
                                nc.vector.tensor_copy(dst, tp[:, :, :])
                            k += 1

                    # softmaxes (row softmax of E and of E^T)
                    zs, rs = [], []
                    for eps in (e1, e2):
                        m_ = smp.tile([128, 1], F32, tag="m")
                        nc.vector.tensor_reduce(m_[:], eps[:], axis=AX.X,
                                                op=ALU.max, negate=True)
                        z_ = smp.tile([128, 128], F16, tag="z")
                        nc.scalar.activation(z_[:], eps[:], AF.Exp, bias=m_[:])
                        s_ = smp.tile([128, 1], F32, tag="s")
                        nc.vector.tensor_reduce(s_[:], z_[:], axis=AX.X,
                                                op=ALU.add)
                        r_ = smp.tile([128, 1], F32, tag="r")
                        nc.vector.reciprocal(r_[:], s_[:])
                        zs.append(z_)
                        rs.append(r_)
                    return zs, rs, big

                def stage_b(n, zs, rs, big):
                    # z transposes + align matmuls + enhancement + out DMA
                    for sd, (z_, othr) in enumerate(((zs[0], big[1]),
                                                     (zs[1], big[0]))):
                        tp = tp_pool.tile([128, 128], F16, tag="tp", name="tp")
                        nc.tensor.transpose(tp[:], z_[:], ident16[:])
                        zt = ztp.tile([128, 128], F16, tag="zt")
                        nc.scalar.activation(zt[:], tp[:], AF.Copy)
                        t_ps = tps_pool.tile([128, H2], F32, tag="tps",
                                             name="tps")
                        for hf in range(2):
                            sl = slice(512 * hf, 512 * (hf + 1))
                            nc.tensor.matmul(t_ps[:, sl], zt[:], othr[:, sl],
                                             start=True, stop=True)
                        b_ = big[sd]
                        r_ = rs[sd]
                        til = b_[:, H2:2 * H2]
                        if sd == 0:
                            nc.scalar.activation(til, t_ps[:], AF.Copy,
                                                 scale=r_[:])
                        else:
                            nc.vector.tensor_scalar_mul(til, t_ps[:], r_[:])
                        outd = outA_d if sd == 0 else outB_d
                        nc.sync.dma_start(outd.ap()[n, :, :], b_[:])

                # software pipeline: stage_b runs two items behind stage_a so
                # its serial tail (zt->align->til->diff/prod->DMA) never sets
                # the loop period
                from collections import deque
                pend = deque()
                for n in range(PB):
                    pend.append((n, *stage_a(n)))
                    if len(pend) > 2:
                        stage_b(*pend.popleft())
                while pend:
                    stage_b(*pend.popleft())

    nc.compile()
    return nc


def _get_nc():
    if "nc" not in _CACHE:
        _CACHE["nc"] = _build()
    return _CACHE["nc"]


def prep_in_maps(inputs):
    A = np.asarray(inputs["A"])
    B = np.asarray(inputs["B"])
    embed = np.asarray(inputs["embed"], dtype=np.float32)
    # permute pytorch gate order [i,f,g,o] -> [f,i,o,g] (all sigmoids
    # contiguous so one activation covers them)
    perm = np.concatenate([np.arange(H, 2 * H), np.arange(0, H),
                           np.arange(3 * H, 4 * H), np.arange(2 * H, 3 * H)])
    wmat = {}
    for d in "fb":
        suf = "_" + d
        wih = np.asarray(inputs["Wih" + suf], dtype=np.float32)[perm]
        whh = np.asarray(inputs["Whh" + suf], dtype=np.float32)[perm]
        bias = (np.asarray(inputs["bih" + suf], dtype=np.float32)
                + np.asarray(inputs["bhh" + suf], dtype=np.float32))[perm]
        w = np.empty((E + 1 + H, G4), dtype=np.float16)
        w[0:E] = wih.T.astype(np.float16)
        w[E] = bias.astype(np.float16)
        w[E + 1:] = whh.T.astype(np.float16)
        wmat[d] = w

    xa = embed[A]    # [BSZ, T, E]
    xb = embed[B]

    in_maps = []
    for c in range(NCORES):
        sl = slice(PB * c, PB * (c + 1))
        xc = np.concatenate([xa[sl], xb[sl]], axis=0)          # [RW, T, E]
        xT = np.empty((E + 1, RW * T), dtype=np.float16)
        xT[0:E] = xc.transpose(2, 0, 1).reshape(E, RW * T).astype(np.float16)
        xT[E] = 1.0
        in_maps.append({
            "xT": xT, "w_f": wmat["f"], "w_b": wmat["b"],
        })
    return in_maps


def kernel(**inputs):
    from concourse.bass_utils import run_bass_kernel_spmd

    in_maps = prep_in_maps(inputs)
    nc = _get_nc()
    res = run_bass_kernel_spmd(nc, in_maps, core_ids=list(range(NCORES)))

    def assemble(name):
        bt = np.concatenate(
            [res.results[c][name].astype(np.float32) for c in range(NCORES)],
            axis=0)                                    # [BSZ, T, 2*H2]
        bar, til = bt[:, :, 0:H2], bt[:, :, H2:2 * H2]
        return np.concatenate([bar, til, bar - til, bar * til], axis=2)

    return assemble("outA"), assemble("outB")


# revision 43
# speedup vs baseline: 3.3830x; 1.0036x over previous
"""Bass/Trainium2 kernel for nn_Encoder (embedding -> BiLSTM -> cross attention
-> enhancement).

Sharding: data-parallel over batch, 16 items per core on 8 NeuronCores (no
collectives). Per core the A and B sequences are stacked into 32 batch rows.

Layout: everything transposed vs the torch reference. Gates are computed as
g^T[2048, 32] = W_ih @ x_t^T + W_hh @ h^T with the gate dimension on SBUF
partitions and the 32 batch rows on the matmul moving dim, so every matmul
streams fp16 at 1 cycle/row and every vector/scalar op runs on all 128
partitions. The x-projection is inlined into the scan (3 extra fp16 K-chunks
per gate chunk; bias rides along as a ones-row of x), and is emitted XAHEAD
steps ahead of the recurrent matmuls to keep the PE array continuously busy
(full p-state). Each step+direction's PSUM bank is one accumulation group:
start=True on the bank's first (prefilled) matmul pend-zeroes the whole 2KB
region, stop=True on the last recurrent matmul. h_t is written by the DVE
directly into a persistent fp16 history tile [128, T, 4, 32] per direction,
which serves as (a) the next step's matmul rhs, (b) the feature-major
attention operands via strided APs, and (c) the source for the time-major
tiles (4 PE transposes batched per PSUM bank, one wide Act/DVE copy out).
The attention items run as a 2-deep software pipeline (E-matmuls+softmax two
items ahead of the z-transpose/align/output stage). The device ships
[bar | til] per item in one fp16 DMA; the host widens to f32 and derives the
bar-til / bar*til enhancement quarters.
"""

import numpy as np

V, E, H = 32000, 300, 512
BSZ, T = 128, 128
NCORES = 8
PB = BSZ // NCORES          # 16 batch items per core per sequence
RW = 2 * PB                 # 32 stacked rows (A items then B items)
G4 = 4 * H                  # 2048 gate width
H2 = 2 * H                  # 1024 bilstm output width
EA = 384                    # padded x feature dim (300 + bias row + zeros)
XC = 3                      # x K-chunks of 128
HC = 4                      # h K-chunks of 128
XAHEAD = 3                  # steps of x-projection prefill ahead of the scan

_CACHE = {}


def _build():
    import concourse.mybir as mybir
    import concourse.tile as tile
    from concourse import bacc
    from concourse.masks import make_identity

    F32 = mybir.dt.float32
    F16 = mybir.dt.float16
    AF = mybir.ActivationFunctionType
    ALU = mybir.AluOpType
    AX = mybir.AxisListType

    nc = bacc.Bacc("TRN2", target_bir_lowering=False, debug=False,
                   num_devices=NCORES)

    # Host-prepped inputs (see prep_in_maps): x^T with ones row for the bias,
    # and per-direction weights stacked as K-chunks (wih+bias rows 0:301,
    # then whh).  Zero padding rows are not shipped: the third x K-chunk is
    # only 45 rows (300 features + ones row - 256).
    KL = E + 1 - 2 * 128  # 45
    xT_d = nc.dram_tensor("xT", [2 * 128 + KL, RW * T], F16,
                          kind="ExternalInput")
    w_d = {d: nc.dram_tensor(f"w_{d}", [2 * 128 + KL + 4 * 128, G4], F16,
                             kind="ExternalInput")
           for d in "fb"}
    # device ships [bar | til] per item; host derives bar-til and bar*til
    outA_d = nc.dram_tensor("outA", [PB, T, 2 * H2], F16, kind="ExternalOutput")
    outB_d = nc.dram_tensor("outB", [PB, T, 2 * H2], F16, kind="ExternalOutput")

    with tile.TileContext(nc) as tc:
        with tc.tile_pool(name="const", bufs=1) as const, \
             tc.tile_pool(name="hist", bufs=1) as histp:
            ident = const.tile([128, 128], F32)
            make_identity(nc, ident[:])
            ident16 = const.tile([128, 128], F16)
            nc.vector.tensor_copy(ident16[:], ident[:])

            # persistent fp16 h history, [128 d-in-chunk, T, h-chunk, batch]
            hist = {d: histp.tile([128, T, HC, RW], F16, name=f"hist_{d}")
                    for d in "fb"}

            # ---------------- Phase 1+2: fused projection + scan ----------
            with tc.tile_pool(name="wst", bufs=1) as wst, \
                 tc.tile_pool(name="xst", bufs=1) as xst, \
                 tc.tile_pool(name="cst", bufs=1) as cst, \
                 tc.tile_pool(name="gct", bufs=2) as gct, \
                 tc.tile_pool(name="ew", bufs=2) as ew, \
                 tc.tile_pool(name="gps", bufs=XAHEAD + 1, space="PSUM") as gps:
                # per-chunk loads ordered so the first x-matmuls can start
                # after just the first chunk of x and fwd weights
                wsb = {d: wst.tile([128, 7, G4], F16, name=f"w_{d}")
                       for d in "fb"}
                xk = xst.tile([128, XC, RW, T], F16, name="xk")
                xlen = [128, 128, KL]
                for k in range(XC):
                    o = k * 128
                    nc.sync.dma_start(
                        xk[0:xlen[k], k, :, :],
                        xT_d.ap()[o:o + xlen[k], :])
                    for d in "fb":
                        nc.sync.dma_start(
                            wsb[d][0:xlen[k], k, :],
                            w_d[d].ap()[o:o + xlen[k], :])
                for k in range(XC, 7):
                    o = 2 * 128 + KL + (k - XC) * 128
                    for d in "fb":
                        nc.sync.dma_start(
                            wsb[d][:, k, :],
                            w_d[d].ap()[o:o + 128, :])
                c_st = {d: cst.tile([128, HC, RW], F16, name=f"c_{d}")
                        for d in "fb"}

                ps_tiles = {}

                # One accumulation group per PSUM bank per step: start=True on
                # the bank's first matmul marks the whole 2KB zero region, so
                # every gate chunk's first write lands zeroed; stop=True on
                # the bank's final matmul.  This keeps the x-projection
                # prefill (emitted XAHEAD steps early) in the same group as
                # the recurrent matmuls without interleaved-group violations.
                def emit_x(d, s):
                    tx = s if d == "f" else T - 1 - s
                    ps = gps.tile([128, 16, RW], F32, tag=f"ps{d}",
                                  name=f"ps{d}")
                    ps_tiles[(d, s)] = ps
                    last = s == 0
                    for gc in range(16):
                        for kx in range(XC):
                            kk = 128 if kx < 2 else KL
                            nc.tensor.matmul(
                                ps[:, gc, :],
                                wsb[d][0:kk, kx, gc * 128:(gc + 1) * 128],
                                xk[0:kk, kx, :, tx],
                                start=(gc == 0 and kx == 0),
                                stop=(last and gc == 15 and kx == XC - 1))

                def emit_h(d, s):
                    tprev = (s - 1) if d == "f" else (T - s)
                    ps = ps_tiles[(d, s)]
                    hp = hist[d][:, tprev, :, :]
                    for gc in range(16):
                        for kc in range(HC):
                            nc.tensor.matmul(
                                ps[:, gc, :],
                                wsb[d][:, XC + kc, gc * 128:(gc + 1) * 128],
                                hp[:, kc, :],
                                start=False,
                                stop=(gc == 15 and kc == HC - 1))

                # Gate chunk layout (host-permuted): [f | i | o | g], 4 chunks
                # of 128 gate dims each; sigmoids contiguous in chunks 0-11.
                for s in range(min(XAHEAD, T)):
                    emit_x("f", s)
                    emit_x("b", s)
                for s in range(T):
                    da, db = ("f", "b") if s % 2 == 0 else ("b", "f")
                    if s >= 1:
                        emit_h(da, s)
                        emit_h(db, s)
                    if s + XAHEAD < T:
                        emit_x(da, s + XAHEAD)
                        emit_x(db, s + XAHEAD)
                    # Activation-queue order tuned so the f-chain's tanh(c)
                    # is not stuck behind the b-dir gate activations, and the
                    # b-chain exploits its natural slack (its matmuls sit in
                    # the second half of each PE burst).
                    ga, th_t = {}, {}
                    for d in "fb":
                        ps = ps_tiles.pop((d, s))
                        g = gct.tile([128, 16, RW], F16, tag=f"ga{d}",
                                     name=f"ga{d}")
                        ga[d] = (g, ps)
                        th_t[d] = ew.tile([128, HC, RW], F16, tag=f"th{d}",
                                          name=f"th{d}")

                    def act_sfi(d):
                        g, ps = ga[d]
                        nc.scalar.activation(g[:, 0:12, :], ps[:, 0:12, :],
                                             AF.Sigmoid)          # f, i, o

                    def act_tg(d):
                        g, ps = ga[d]
                        nc.scalar.activation(g[:, 12:16, :], ps[:, 12:16, :],
                                             AF.Tanh)             # g

                    def act_so(d):
                        pass

                    def act_tc(d):
                        nc.scalar.activation(th_t[d][:], c_st[d][:], AF.Tanh)

                    def dve_q(d):
                        if s == 0:
                            return None
                        g, _ = ga[d]
                        q_ = ew.tile([128, HC, RW], F16, tag=f"q{d}",
                                     name=f"q{d}")
                        nc.vector.tensor_mul(q_[:], g[:, 0:4, :], c_st[d][:])
                        return q_

                    def dve_pc(d, q_):
                        g, _ = ga[d]
                        if s == 0:
                            nc.vector.tensor_mul(c_st[d][:], g[:, 4:8, :],
                                                 g[:, 12:16, :])  # c = i*g
                        else:
                            p_ = ew.tile([128, HC, RW], F16, tag=f"p{d}",
                                         name=f"p{d}")
                            nc.vector.tensor_mul(p_[:], g[:, 4:8, :],
                                                 g[:, 12:16, :])  # i*g
                            nc.vector.tensor_add(c_st[d][:], p_[:], q_[:])

                    def dve_h(d):
                        tx = s if d == "f" else T - 1 - s
                        nc.vector.tensor_mul(hist[d][:, tx, :, :],
                                             ga[d][0][:, 8:12, :], th_t[d][:])

                    # Alternate which direction leads each step so the act
                    # queue penalty of going second is shared evenly.
                    d1, d2 = ("f", "b") if s % 2 == 0 else ("b", "f")
                    act_sfi(d1)
                    act_tg(d1)
                    act_so(d1)
                    q1 = dve_q(d1)
                    dve_pc(d1, q1)
                    act_sfi(d2)
                    act_tc(d1)
                    q2 = dve_q(d2)
                    dve_h(d1)
                    act_tg(d2)
                    act_so(d2)
                    dve_pc(d2, q2)
                    act_tc(d2)
                    dve_h(d2)

            # ---------------- Phase 3: attention + enhancement -------------
            with tc.tile_pool(name="big", bufs=5) as bigp, \
                 tc.tile_pool(name="sm", bufs=6) as smp, \
                 tc.tile_pool(name="zt", bufs=4) as ztp, \
                 tc.tile_pool(name="eps", bufs=2, space="PSUM") as eps_pool, \
                 tc.tile_pool(name="tp", bufs=4, space="PSUM") as tp_pool, \
                 tc.tile_pool(name="tps", bufs=1, space="PSUM") as tps_pool:

                def fm(side, c8, n):
                    # feature-major [128 d, 128 t] strided view of history
                    d = "f" if c8 < 4 else "b"
                    return hist[d][:, :, c8 % 4, n if side == 0 else PB + n]

                def stage_a(n):
                    # E matmuls straight off the history (strided APs);
                    # e1/e2 pack into one PSUM bank
                    e12 = eps_pool.tile([128, 2, 128], F32, tag="e", name="e12")
                    e1, e2 = e12[:, 0, :], e12[:, 1, :]
                    for c8 in range(8):
                        nc.tensor.matmul(e1, fm(0, c8, n), fm(1, c8, n),
                                         start=(c8 == 0), stop=(c8 == 7))
                    for c8 in range(8):
                        nc.tensor.matmul(e2, fm(1, c8, n), fm(0, c8, n),
                                         start=(c8 == 0), stop=(c8 == 7))

                    # time-major tiles: 4 PE transposes batched per PSUM bank
                    # (one pending-zero group), then a single wide copy per
                    # bank alternating Act/DVE (GPSIMD cannot read PSUM).
                    # big layout: [bar | til]
                    big = [bigp.tile([128, 2 * H2], F16, tag=f"big{sd}",
                                     name=f"big{sd}") for sd in range(2)]
                    k = 0
                    for sd in range(2):
                        for half in range(2):
                            tp = tp_pool.tile([128, 4, 128], F16, tag="tp",
                                              name="tp")
                            for j in range(4):
                                nc.tensor.matmul(
                                    tp[:, j, :], fm(sd, half * 4 + j, n),
                                    ident16[:], is_transpose=True,
                                    start=(j == 0), stop=(j == 3))
                            dst = big[sd][:, half * 512:(half + 1) * 512]
                            if k % 2 == 0:
                                nc.scalar.activation(dst, tp[:, :, :], AF.Copy)
                            else:
                                nc.vector.tensor_copy(dst, tp[:, :, :])
                            k += 1

                    # softmaxes (row softmax of E and of E^T)
                    zs, rs = [], []
                    for eps in (e1, e2):
                        m_ = smp.tile([128, 1], F32, tag="m")
                        nc.vector.tensor_reduce(m_[:], eps[:], axis=AX.X,
                                                op=ALU.max, negate=True)
                        z_ = smp.tile([128, 128], F16, tag="z")
                        nc.scalar.activation(z_[:], eps[:], AF.Exp, bias=m_[:])
                        s_ = smp.tile([128, 1], F32, tag="s")
                        nc.vector.tensor_reduce(s_[:], z_[:], axis=AX.X,
                                                op=ALU.add)
                        r_ = smp.tile([128, 1], F32, tag="r")
                        nc.vector.reciprocal(r_[:], s_[:])
                        zs.append(z_)
                        rs.append(r_)
                    return zs, rs, big

                def stage_b(n, zs, rs, big):
                    # z transposes + align matmuls + enhancement + out DMA
                    for sd, (z_, othr) in enumerate(((zs[0], big[1]),
                                                     (zs[1], big[0]))):
                        tp = tp_pool.tile([128, 128], F16, tag="tp", name="tp")
                        nc.tensor.transpose(tp[:], z_[:], ident16[:])
                        zt = ztp.tile([128, 128], F16, tag="zt")
                        nc.scalar.activation(zt[:], tp[:], AF.Copy)
                        t_ps = tps_pool.tile([128, H2], F32, tag="tps",
                                             name="tps")
                        for hf in range(2):
                            sl = slice(512 * hf, 512 * (hf + 1))
                            nc.tensor.matmul(t_ps[:, sl], zt[:], othr[:, sl],
                                             start=True, stop=True)
                        b_ = big[sd]
                        r_ = rs[sd]
                        til = b_[:, H2:2 * H2]
                        if sd == 0:
                            nc.scalar.activation(til, t_ps[:], AF.Copy,
                                                 scale=r_[:])
                        else:
                            nc.vector.tensor_scalar_mul(til, t_ps[:], r_[:])
                        outd = outA_d if sd == 0 else outB_d
                        nc.sync.dma_start(outd.ap()[n, :, :], b_[:])

                # software pipeline: stage_b runs two items behind stage_a so
                # its serial tail (zt->align->til->diff/prod->DMA) never sets
                # the loop period
                from collections import deque
                pend = deque()
                for n in range(PB):
                    pend.append((n, *stage_a(n)))
                    if len(pend) > 2:
                        stage_b(*pend.popleft())
                while pend:
                    stage_b(*pend.popleft())

    nc.compile()
    return nc


def _get_nc():
    if "nc" not in _CACHE:
        _CACHE["nc"] = _build()
    return _CACHE["nc"]


def prep_in_maps(inputs):
    A = np.asarray(inputs["A"])
    B = np.asarray(inputs["B"])
    embed = np.asarray(inputs["embed"], dtype=np.float32)
    # permute pytorch gate order [i,f,g,o] -> [f,i,o,g] (all sigmoids
    # contiguous so one activation covers them)
    perm = np.concatenate([np.arange(H, 2 * H), np.arange(0, H),
                           np.arange(3 * H, 4 * H), np.arange(2 * H, 3 * H)])
    wmat = {}
    for d in "fb":
        suf = "_" + d
        wih = np.asarray(inputs["Wih" + suf], dtype=np.float32)[perm]
        whh = np.asarray(inputs["Whh" + suf], dtype=np.float32)[perm]
        bias = (np.asarray(inputs["bih" + suf], dtype=np.float32)
                + np.asarray(inputs["bhh" + suf], dtype=np.float32))[perm]
        w = np.empty((E + 1 + H, G4), dtype=np.float16)
        w[0:E] = wih.T.astype(np.float16)
        w[E] = bias.astype(np.float16)
        w[E + 1:] = whh.T.astype(np.float16)
        wmat[d] = w

    xa = embed[A]    # [BSZ, T, E]
    xb = embed[B]

    in_maps = []
    for c in range(NCORES):
        sl = slice(PB * c, PB * (c + 1))
        xc = np.concatenate([xa[sl], xb[sl]], axis=0)          # [RW, T, E]
        xT = np.empty((E + 1, RW * T), dtype=np.float16)
        xT[0:E] = xc.transpose(2, 0, 1).reshape(E, RW * T).astype(np.float16)
        xT[E] = 1.0
        in_maps.append({
            "xT": xT, "w_f": wmat["f"], "w_b": wmat["b"],
        })
    return in_maps


def kernel(**inputs):
    from concourse.bass_utils import run_bass_kernel_spmd

    in_maps = prep_in_maps(inputs)
    nc = _get_nc()
    res = run_bass_kernel_spmd(nc, in_maps, core_ids=list(range(NCORES)))

    def assemble(name):
        bt = np.concatenate(
            [res.results[c][name].astype(np.float32) for c in range(NCORES)],
            axis=0)                                    # [BSZ, T, 2*H2]
        bar, til = bt[:, :, 0:H2], bt[:, :, H2:2 * H2]
        return np.concatenate([bar, til, bar - til, bar * til], axis=2)

    return assemble("outA"), assemble("outB")
